# revision 1
# baseline (speedup 1.0000x reference)
"""Trainium2 Bass kernel for nn_ExaoneAttention (dense transformer attention).

Full-input contract: kernel(**inputs) takes the unsharded inputs and returns
the full [B, S, D] output. Internally shards across 8 NeuronCores:
2-way data parallel over batch x 4-way tensor parallel over kv heads
(2 kv heads = 8 query heads per core). Each core computes a partial
output through its Wo row-slice; the host sums the 4 partials per batch.

All matmuls run in float32r (full PE rate, ~1e-4 relative rounding).
Attention is computed in the "scoresT" orientation (keys on partitions,
queries on the free dim) so probs feed the PV matmul with no transposes;
softmax normalization uses a ones-vector matmul partition-reduce plus a
rank-1 broadcast matmul.
"""

import contextlib
import ctypes
import os
import sys
import types

import numpy as np

# ---------------------------------------------------------------------------
# Problem constants (hardcoded per contract)
# ---------------------------------------------------------------------------
B, S, D = 2, 2048, 4096
H, HKV, HD = 32, 8, 128
G = H // HKV
THETA = 10000.0

NCORES = 8
BAT_SHARDS = 2
KV_SHARDS = 4
KVH = HKV // KV_SHARDS  # kv heads per core = 2
QH = KVH * G  # q heads per core = 8
FQ = QH * HD  # 1024
FKV = KVH * HD  # 256
DMC = D // 128  # 32 model-dim chunks

QT = 512  # query tile
NQT = S // QT  # 4
SC = 128  # sequence chunk
NSC = S // SC  # 16
DT = 512  # output d tile
NDT = D // DT  # 8

_SCALE = float(HD) ** -0.5


# ---------------------------------------------------------------------------
# Wait-count legalization: this walrus build rejects instructions carrying
# more than a small number of sync waits (fused fp32/fp32r matmul: >1;
# drain: >4). Hoist excess waits onto standalone NoOps on the same engine
# immediately before the offending instruction; AND-semantics are preserved
# by sequential same-engine execution.
# ---------------------------------------------------------------------------
def _legalize_waits(nc):
    import bass_rust
    import concourse.mybir as mybir

    counter = 0
    for f in nc.m.functions:
        for bb in f.blocks:
            il = bb.instructions
            i = 0
            while i < len(il):
                ins = il[i]
                si = ins.sync_info
                if si is None or len(si.on_wait) <= 1:
                    i += 1
                    continue
                waits = list(si.on_wait)
                pos = i
                for w in waits[1:]:
                    counter += 1
                    nop = mybir.InstNoOp(name=f"lgw-{counter}", ins=[], outs=[])
                    nop.engine = ins.engine
                    nop.sync_info = bass_rust.SyncInfo(on_wait=[w], on_update=[])
                    il.insert(pos, nop)
                    pos += 1
                    i += 1
                ins.sync_info = bass_rust.SyncInfo(
                    on_wait=waits[:1], on_update=list(si.on_update)
                )
                i += 1
    return counter


# ---------------------------------------------------------------------------
# Bass kernel builder (per-core program; same program on all 8 cores)
# ---------------------------------------------------------------------------
def _build_nc():
    import concourse.bass as bass
    import concourse.mybir as mybir
    from concourse.masks import make_identity
    from concourse.tile import TileContext

    f32 = mybir.dt.float32
    f32r = mybir.dt.float32r
    AF = mybir.ActivationFunctionType

    nc = bass.Bass()

    hiT = nc.declare_dram_parameter("hiT", [D, S], f32, isOutput=False)
    wq = nc.declare_dram_parameter("wq", [D, FQ], f32, isOutput=False)
    wk = nc.declare_dram_parameter("wk", [D, FKV], f32, isOutput=False)
    wv = nc.declare_dram_parameter("wv", [D, FKV], f32, isOutput=False)
    wo = nc.declare_dram_parameter("wo", [FQ, D], f32, isOutput=False)
    ccT = nc.declare_dram_parameter("ccT", [HD, S], f32, isOutput=False)
    ssT = nc.declare_dram_parameter("ssT", [HD, S], f32, isOutput=False)
    dmask = nc.declare_dram_parameter("dmask", [SC, G * QT], f32, isOutput=False)
    out = nc.declare_dram_parameter("out", [S, D], f32, isOutput=True)

    # internal DRAM staging
    qT_d = nc.dram_tensor("qT_d", [QH, HD, S], f32)
    kT_d = nc.dram_tensor("kT_d", [KVH, HD, S], f32)
    v_d = nc.dram_tensor("v_d", [KVH, S, HD], f32)
    ctxT_d = nc.dram_tensor("ctxT_d", [QH, HD, S], f32)

    hiT_r = hiT[:, :].bitcast(f32r).rearrange("(c p) s -> p c s", p=128)
    wq_r = wq[:, :].bitcast(f32r).rearrange("(c p) f -> p c f", p=128)
    wk_r = wk[:, :].bitcast(f32r).rearrange("(c p) f -> p c f", p=128)
    wv_r = wv[:, :].bitcast(f32r).rearrange("(c p) f -> p c f", p=128)
    wo_r = wo[:, :].bitcast(f32r).rearrange("(h p) d -> p h d", p=128)

    def rope(vec, out_sb, psum, cc, ss):
        """out = rope(psum) elementwise with cc/ss [128, W] tables."""
        t1 = rope_tmp_pool.tile(list(out_sb.shape), f32, name="rope_t1")
        vec.tensor_mul(t1, psum, cc)
        t2 = rope_tmp_pool.tile(list(out_sb.shape), f32, name="rope_t2")
        vec.tensor_mul(t2[:64], psum[64:], ss[:64])
        vec.tensor_mul(t2[64:], psum[:64], ss[64:])
        vec.tensor_sub(out_sb[:64], t1[:64], t2[:64])
        vec.tensor_add(out_sb[64:], t1[64:], t2[64:])

    with TileContext(nc) as tc, contextlib.ExitStack() as top:
        singles = top.enter_context(tc.tile_pool(name="singles", bufs=1))
        rope_tmp_pool = top.enter_context(tc.tile_pool(name="ropetmp", bufs=2))

        cc_sb = singles.tile([HD, S], f32)
        nc.sync.dma_start(out=cc_sb, in_=ccT[:, :])
        ss_sb = singles.tile([HD, S], f32)
        nc.sync.dma_start(out=ss_sb, in_=ssT[:, :])
        dm_sb = singles.tile([SC, G * QT], f32)
        nc.sync.dma_start(out=dm_sb, in_=dmask[:, :])
        ident = singles.tile([128, 128], f32)
        make_identity(nc, ident)
        ones_tmp = singles.tile([128, 128], f32)
        nc.vector.memset(ones_tmp, 1.0)
        ones_p = singles.tile([128, 1], f32r)
        nc.vector.tensor_copy(ones_p, ones_tmp[:, :1])
        ones_f = singles.tile([1, 128], f32r)
        nc.vector.tensor_copy(ones_f, ones_tmp[:1, :])

        # ---------------- Phase A: K/V projection (+rope K, transpose V) ----
        with contextlib.ExitStack() as ph:
            wkv_pool = ph.enter_context(tc.tile_pool(name="wkv", bufs=1))
            hi_pool = ph.enter_context(tc.tile_pool(name="hiA", bufs=2))
            stage_pool = ph.enter_context(tc.tile_pool(name="stageA", bufs=3))
            psA = ph.enter_context(tc.tile_pool(name="psA", bufs=4, space="PSUM"))
            psT = ph.enter_context(tc.tile_pool(name="psTr", bufs=2, space="PSUM"))

            wk_sb = wkv_pool.tile([128, DMC, FKV], f32r)
            nc.sync.dma_start(out=wk_sb, in_=wk_r)
            wv_sb = wkv_pool.tile([128, DMC, FKV], f32r)
            nc.sync.dma_start(out=wv_sb, in_=wv_r)

            for st in range(NQT):
                ssl = slice(st * QT, (st + 1) * QT)
                # two half-slabs of hiT for this s-tile (SBUF headroom)
                slabs = []
                for hh in range(2):
                    slab = hi_pool.tile([128, DMC // 2, QT], f32r, name="hiA_slab")
                    nc.sync.dma_start(
                        out=slab, in_=hiT_r[:, hh * (DMC // 2) : (hh + 1) * (DMC // 2), ssl]
                    )
                    slabs.append(slab)

                for fc in range(2 * KVH):  # k0,k1,v0,v1
                    is_k = fc < KVH
                    w_sb = wk_sb if is_k else wv_sb
                    fs = slice((fc % KVH) * 128, (fc % KVH) * 128 + 128)
                    pk = psA.tile([128, QT], f32, name="psA")
                    for c in range(DMC):
                        nc.tensor.matmul(
                            pk,
                            w_sb[:, c, fs],
                            slabs[c // (DMC // 2)][:, c % (DMC // 2), :],
                            start=(c == 0),
                            stop=(c == DMC - 1),
                        )
                    kv = fc % KVH
                    if is_k:
                        kt_sb = stage_pool.tile([128, QT], f32r, name="kt_st")
                        rope(nc.vector, kt_sb, pk, cc_sb[:, ssl], ss_sb[:, ssl])
                        nc.sync.dma_start(out=kT_d[kv, :, ssl].bitcast(f32r), in_=kt_sb)
                    else:
                        vt_st = stage_pool.tile([128, QT], f32, name="vt_st")
                        nc.scalar.copy(vt_st, pk)
                        for j in range(QT // 128):
                            ptr = psT.tile([128, 128], f32, name="ptr")
                            nc.tensor.transpose(ptr, vt_st[:, j * 128 : (j + 1) * 128], ident)
                            vblk = stage_pool.tile([128, 128], f32, name="vblk")
                            nc.scalar.copy(vblk, ptr)
                            s0 = st * QT + j * 128
                            nc.sync.dma_start(out=v_d[kv, s0 : s0 + 128, :], in_=vblk)

        # ---------------- Phase B: Q projection (+rope) ---------------------
        with contextlib.ExitStack() as ph:
            wq_pool = ph.enter_context(tc.tile_pool(name="wqp", bufs=1))
            hi_pool = ph.enter_context(tc.tile_pool(name="hiB", bufs=3))
            stage_pool = ph.enter_context(tc.tile_pool(name="stageB", bufs=3))
            psB = ph.enter_context(tc.tile_pool(name="psB", bufs=8, space="PSUM"))

            wq_sb = wq_pool.tile([128, DMC, FQ], f32r)
            nc.sync.dma_start(out=wq_sb, in_=wq_r)

            for qt in range(NQT):
                ssl = slice(qt * QT, (qt + 1) * QT)
                banks = [psB.tile([128, QT], f32, name="psB") for _ in range(QH)]
                for c in range(DMC):
                    hi_t = hi_pool.tile([128, QT], f32r, name="hiB_t")
                    nc.sync.dma_start(out=hi_t, in_=hiT_r[:, c, ssl])
                    for h in range(QH):
                        nc.tensor.matmul(
                            banks[h],
                            wq_sb[:, c, h * 128 : (h + 1) * 128],
                            hi_t,
                            start=(c == 0),
                            stop=(c == DMC - 1),
                        )
                for h in range(QH):
                    qt_sb = stage_pool.tile([128, QT], f32r, name="qt_st")
                    rope(nc.vector, qt_sb, banks[h], cc_sb[:, ssl], ss_sb[:, ssl])
                    nc.sync.dma_start(out=qT_d[h, :, ssl].bitcast(f32r), in_=qt_sb)

        # ---------------- Phase C: attention --------------------------------
        with contextlib.ExitStack() as ph:
            kv_pool = ph.enter_context(tc.tile_pool(name="kvp", bufs=1))
            q_pool = ph.enter_context(tc.tile_pool(name="qp", bufs=3))
            pt_pool = ph.enter_context(tc.tile_pool(name="ptp", bufs=3))
            acc_pool = ph.enter_context(tc.tile_pool(name="accp", bufs=2))
            misc_pool = ph.enter_context(tc.tile_pool(name="miscC", bufs=3))
            ps_s = ph.enter_context(tc.tile_pool(name="ps_s", bufs=3, space="PSUM"))
            ps_ctx = ph.enter_context(tc.tile_pool(name="ps_ctx", bufs=2, space="PSUM"))
            ps_r = ph.enter_context(tc.tile_pool(name="ps_r", bufs=1, space="PSUM"))

            kt_sb = kv_pool.tile([128, KVH, S], f32r)
            nc.sync.dma_start(out=kt_sb, in_=kT_d[:, :, :].bitcast(f32r).rearrange("k p s -> p k s"))
            v_sb = kv_pool.tile([128, KVH, NSC, HD], f32r)
            nc.sync.dma_start(
                out=v_sb,
                in_=v_d[:, :, :].bitcast(f32r).rearrange("k (sc p) d -> p k sc d", p=128),
            )

            for qt in range(NQT):
                ssl = slice(qt * QT, (qt + 1) * QT)
                nk = G * (qt + 1)
                for h in range(QH):
                    kv = h // G
                    qt_sb = q_pool.tile([128, QT], f32r, name="qt_at")
                    nc.sync.dma_start(out=qt_sb, in_=qT_d[h, :, ssl].bitcast(f32r))
                    pctx = ps_ctx.tile([128, QT], f32, name="pctx")
                    acc = acc_pool.tile([128, QT], f32r, name="acc")
                    for i in range(nk):
                        pss = ps_s.tile([128, QT], f32, name="pss")
                        nc.tensor.matmul(
                            pss,
                            kt_sb[:, kv, i * 128 : (i + 1) * 128],
                            qt_sb,
                            start=True,
                            stop=True,
                        )
                        if i >= G * qt:
                            t = i - G * qt
                            nc.vector.tensor_add(
                                pss, pss, dm_sb[:, t * QT : (t + 1) * QT]
                            )
                        pt = pt_pool.tile([128, QT], f32r, name="pt")
                        nc.scalar.activation(pt, pss, AF.Exp, scale=_SCALE)
                        nc.tensor.matmul(
                            pctx,
                            v_sb[:, kv, i, :],
                            pt,
                            start=(i == 0),
                            stop=(i == nk - 1),
                        )
                        if i == 0:
                            nc.vector.tensor_copy(acc, pt)
                        else:
                            nc.vector.tensor_add(acc, acc, pt)
                    pred = ps_r.tile([1, QT], f32, name="pred")
                    nc.tensor.matmul(pred, ones_p, acc, start=True, stop=True)
                    recip = misc_pool.tile([1, QT], f32r, name="recip")
                    with nc.allow_low_precision(reason="f32r recip: 1e-4 ok here"):
                        nc.vector.reciprocal(recip, pred)
                    pbc = ps_r.tile([128, QT], f32, name="pbc")
                    nc.tensor.matmul(pbc, ones_f, recip, start=True, stop=True)
                    bc_sb = misc_pool.tile([128, QT], f32, name="bc_sb")
                    nc.scalar.copy(bc_sb, pbc)
                    ctx_sb = misc_pool.tile([128, QT], f32r, name="ctx_sb")
                    nc.vector.tensor_mul(ctx_sb, pctx, bc_sb)
                    nc.sync.dma_start(out=ctxT_d[h, :, ssl].bitcast(f32r), in_=ctx_sb)

        # ---------------- Phase D: output projection ------------------------
        with contextlib.ExitStack() as ph:
            wo_pool = ph.enter_context(tc.tile_pool(name="wop", bufs=1))
            cx_pool = ph.enter_context(tc.tile_pool(name="cxp", bufs=3))
            o_pool = ph.enter_context(tc.tile_pool(name="op", bufs=3))
            ps_o = ph.enter_context(tc.tile_pool(name="ps_o", bufs=3, space="PSUM"))

            wo_sb = wo_pool.tile([128, QH, D], f32r)
            nc.sync.dma_start(out=wo_sb, in_=wo_r)

            for sc in range(NSC):
                cx_sb = cx_pool.tile([128, QH, 128], f32r, name="cx")
                nc.sync.dma_start(
                    out=cx_sb,
                    in_=ctxT_d[:, :, sc * 128 : (sc + 1) * 128]
                    .bitcast(f32r)
                    .rearrange("h p s -> p h s"),
                )
                for dt in range(NDT):
                    po = ps_o.tile([128, DT], f32, name="po")
                    for h in range(QH):
                        nc.tensor.matmul(
                            po,
                            cx_sb[:, h, :],
                            wo_sb[:, h, dt * DT : (dt + 1) * DT],
                            start=(h == 0),
                            stop=(h == QH - 1),
                        )
                    o_sb = o_pool.tile([128, DT], f32, name="o_sb")
                    nc.scalar.copy(o_sb, po)
                    nc.sync.dma_start(
                        out=out[sc * 128 : (sc + 1) * 128, dt * DT : (dt + 1) * DT],
                        in_=o_sb,
                    )

    _legalize_waits(nc)
    return nc


_NC_CACHE = {}
_last_exec_ns = None


def _get_nc():
    if "nc" not in _NC_CACHE:
        _NC_CACHE["nc"] = _build_nc()
    return _NC_CACHE["nc"]


# ---------------------------------------------------------------------------
# Optional NTFF profiling hook (used by the local test harness via
# KERNEL_TRACE=1; grading path leaves it off)
# ---------------------------------------------------------------------------
def _install_ntff_hook(so_path="/opt/axon/libaxon_pjrt.so"):
    if "antenv.axon_hooks" in sys.modules:
        return
    try:
        lib = ctypes.CDLL(so_path)
    except OSError:
        lib = None
    if lib is None or not hasattr(lib, "axon_start_nrt_profile"):
        hook = None
    else:
        lib.axon_start_nrt_profile.argtypes = [
            ctypes.POINTER(ctypes.c_int64),
            ctypes.c_size_t,
        ]
        lib.axon_start_nrt_profile.restype = ctypes.c_int64
        lib.axon_stop_nrt_profile.argtypes = [ctypes.c_char_p]
        lib.axon_stop_nrt_profile.restype = ctypes.c_int64

        @contextlib.contextmanager
        def hook(output_dir, device_ids):
            import jax

            jax.devices()
            if device_ids:
                ids = (ctypes.c_int64 * len(device_ids))(*device_ids)
                rc = lib.axon_start_nrt_profile(ids, len(device_ids))
            else:
                rc = lib.axon_start_nrt_profile(None, 0)
            if rc != 0:
                raise RuntimeError(f"axon_start_nrt_profile rc={rc}")
            try:
                yield
            finally:
                n = lib.axon_stop_nrt_profile(str(output_dir).encode())
                print(f"ntff profile: {n} file(s) -> {output_dir}", file=sys.stderr)

    mod = types.ModuleType("antenv.axon_hooks")
    mod.get_axon_ntff_profile_hook = lambda: hook
    sys.modules["antenv.axon_hooks"] = mod


# ---------------------------------------------------------------------------
# Host entry point
# ---------------------------------------------------------------------------
def kernel(hidden_states, position_ids, attention_mask, Wq, Wk, Wv, Wo):
    global _last_exec_ns
    from concourse import bass_utils

    hidden_states = np.asarray(hidden_states, dtype=np.float32)
    position_ids = np.asarray(position_ids)
    attention_mask = np.asarray(attention_mask)
    Wq = np.asarray(Wq, dtype=np.float32)
    Wk = np.asarray(Wk, dtype=np.float32)
    Wv = np.asarray(Wv, dtype=np.float32)
    Wo = np.asarray(Wo, dtype=np.float32)

    if not np.all(np.asarray(attention_mask) > 0):
        # Spec guarantees an all-ones mask; fall back to a host reference
        # implementation for the general case rather than mis-computing.
        return _host_reference(
            hidden_states, position_ids, attention_mask, Wq, Wk, Wv, Wo
        )

    # rope tables per batch: cc/ss [HD, S] with halves stacked
    half = HD // 2
    inv_freq = 1.0 / (THETA ** (np.arange(0, half, dtype=np.float32) / half))
    ccs, sss = [], []
    for b in range(B):
        freqs = position_ids[b].astype(np.float32)[:, None] * inv_freq[None, :]
        cosT = np.cos(freqs).T.astype(np.float32)  # [64, S]
        sinT = np.sin(freqs).T.astype(np.float32)
        ccs.append(np.ascontiguousarray(np.concatenate([cosT, cosT], axis=0)))
        sss.append(np.ascontiguousarray(np.concatenate([sinT, sinT], axis=0)))

    # causal diagonal masks: block t in [0, G): dmask[kk, t*QT + qq] = 0 if
    # qq >= t*128 + kk else -1e30  (pre-scale additive, exp -> 0)
    kk = np.arange(SC)[:, None]
    qq = np.arange(QT)[None, :]
    dmask = np.concatenate(
        [
            np.where(qq >= t * SC + kk, 0.0, -1.0e30).astype(np.float32)
            for t in range(G)
        ],
        axis=1,
    )
    dmask = np.ascontiguousarray(dmask)

    hiTs = [np.ascontiguousarray(hidden_states[b].T) for b in range(B)]

    in_maps = []
    for c in range(NCORES):
        b = c // KV_SHARDS
        m = c % KV_SHARDS
        qcols = slice(m * FQ, (m + 1) * FQ)
        kvcols = slice(m * FKV, (m + 1) * FKV)
        in_maps.append(
            {
                "hiT": hiTs[b],
                "wq": np.ascontiguousarray(Wq[:, qcols]),
                "wk": np.ascontiguousarray(Wk[:, kvcols]),
                "wv": np.ascontiguousarray(Wv[:, kvcols]),
                "wo": np.ascontiguousarray(Wo[qcols, :]),
                "ccT": ccs[b],
                "ssT": sss[b],
                "dmask": dmask,
            }
        )

    nc = _get_nc()
    trace = os.environ.get("KERNEL_TRACE", "") == "1"
    if trace:
        _install_ntff_hook()
        bass_utils.upload_artifacts = lambda tmpdir: f"local:{tmpdir}"
    res = bass_utils.run_bass_kernel_spmd(
        nc, in_maps, list(range(NCORES)), trace=trace
    )
    _last_exec_ns = res.exec_time_ns

    out = np.zeros((B, S, D), dtype=np.float32)
    for c in range(NCORES):
        out[c // KV_SHARDS] += res.results[c]["out"]
    return out


def _host_reference(hidden_states, position_ids, attention_mask, Wq, Wk, Wv, Wo):
    """Numpy fallback for inputs outside the spec's guarantees."""
    q = (hidden_states @ Wq).reshape(B, S, H, HD)
    k = (hidden_states @ Wk).reshape(B, S, HKV, HD)
    v = (hidden_states @ Wv).reshape(B, S, HKV, HD)

    half = HD // 2
    inv_freq = 1.0 / (THETA ** (np.arange(0, half, dtype=np.float32) / half))
    freqs = position_ids.astype(np.float32)[..., None] * inv_freq
    cos = np.cos(freqs)[:, :, None, :]
    sin = np.sin(freqs)[:, :, None, :]

    def rope(x):
        x1, x2 = x[..., :half], x[..., half:]
        return np.concatenate([x1 * cos - x2 * sin, x2 * cos + x1 * sin], axis=-1)

    q, k = rope(q), rope(k)
    qg = q.reshape(B, S, HKV, G, HD)
    scores = np.einsum("bqhgd,bkhd->bhgqk", qg, k) * (HD**-0.5)
    causal = np.tril(np.ones((S, S), bool))
    mask = causal[None, None, None] & (attention_mask[:, None, None, None, :] > 0)
    scores = np.where(mask, scores, np.finfo(np.float32).min)
    scores = scores - scores.max(axis=-1, keepdims=True)
    probs = np.exp(scores)
    probs = probs / probs.sum(axis=-1, keepdims=True)
    ctx = np.einsum("bhgqk,bkhd->bqhgd", probs, v).reshape(B, S, H * HD)
    return (ctx @ Wo).astype(np.float32)



# revision 7
# speedup vs baseline: 1.4379x; 1.4379x over previous
"""Trainium2 Bass kernel for nn_ExaoneAttention (dense transformer attention).

Full-input contract: kernel(**inputs) takes the unsharded inputs and returns
the full [B, S, D] output. Internally shards across 8 NeuronCores:
2-way data parallel over batch x 4-way tensor parallel over kv heads
(2 kv heads = 8 query heads per core). Each core computes a partial
output through its Wo row-slice; the host sums the 4 partials per batch.

v2 design (bf16 operands, f32 PSUM accumulation):
- Single fused pass: for each 512-token tile, project K/V/Q (weights
  stationary, hidden-state tile moving), rope on the fly, then run
  attention for that query tile against all keys so far. Output
  projection runs as a tail phase with Wo streamed per 512-column block.
  No DRAM staging round-trips; K/V/ctx live in SBUF for the whole kernel.
- Softmax in the "scoresT" orientation (keys on partitions, queries on
  the free dim). Per-chunk exp runs on 1024-wide PSUM pairs. Row sums
  accumulate via matmuls with a head-selector stationary into one
  [8, 512] PSUM tile per query tile, giving a single batched reciprocal;
  the reciprocal row is broadcast back over partitions with a tiny
  matmul and multiplied into the unnormalized context in place.
- Causal diagonal blocks are handled multiplicatively: exp first, then
  a 0/1 bf16 mask multiply (2x DVE throughput).
"""

import contextlib
import ctypes
import os
import sys
import types

import ml_dtypes
import numpy as np

# ---------------------------------------------------------------------------
# Problem constants (hardcoded per contract)
# ---------------------------------------------------------------------------
B, S, D = 2, 2048, 4096
H, HKV, HD = 32, 8, 128
G = H // HKV
THETA = 10000.0

NCORES = 8
BAT_SHARDS = 2
KV_SHARDS = 4
KVH = HKV // KV_SHARDS  # kv heads per core = 2
QH = KVH * G  # q heads per core = 8
FQ = QH * HD  # 1024
FKV = KVH * HD  # 256
DMC = D // 128  # 32 model-dim chunks

QT = 512  # query tile
NQT = S // QT  # 4
SC = 128  # sequence chunk
NSC = S // SC  # 16
DT = 512  # output d tile
NDT = D // DT  # 8

_SCALE = float(HD) ** -0.5
BF16 = ml_dtypes.bfloat16


# ---------------------------------------------------------------------------
# Wait-count legalization: this walrus build rejects instructions carrying
# more than a small number of sync waits (fused fp32/fp32r matmul: >1;
# drain: >4). Hoist excess waits onto standalone NoOps on the same engine
# immediately before the offending instruction; AND-semantics are preserved
# by sequential same-engine execution.
# ---------------------------------------------------------------------------
def _legalize_waits(nc):
    import bass_rust
    import concourse.mybir as mybir

    counter = 0
    for f in nc.m.functions:
        for bb in f.blocks:
            il = bb.instructions
            i = 0
            while i < len(il):
                ins = il[i]
                si = ins.sync_info
                if si is None or len(si.on_wait) <= 1:
                    i += 1
                    continue
                waits = list(si.on_wait)
                pos = i
                for w in waits[1:]:
                    counter += 1
                    nop = mybir.InstNoOp(name=f"lgw-{counter}", ins=[], outs=[])
                    nop.engine = ins.engine
                    nop.sync_info = bass_rust.SyncInfo(on_wait=[w], on_update=[])
                    il.insert(pos, nop)
                    pos += 1
                    i += 1
                ins.sync_info = bass_rust.SyncInfo(
                    on_wait=waits[:1], on_update=list(si.on_update)
                )
                i += 1
    return counter


# ---------------------------------------------------------------------------
# Bass kernel builder (per-core program; same program on all 8 cores)
# ---------------------------------------------------------------------------
def _build_nc():
    import concourse.bass as bass
    import concourse.mybir as mybir
    from concourse.masks import make_identity
    from concourse.tile import TileContext

    f32 = mybir.dt.float32
    bf = mybir.dt.bfloat16
    AF = mybir.ActivationFunctionType

    nc = bass.Bass()

    hiT = nc.declare_dram_parameter("hiT", [D, S], bf, isOutput=False)
    wq = nc.declare_dram_parameter("wq", [D, FQ], bf, isOutput=False)
    wk = nc.declare_dram_parameter("wk", [D, FKV], bf, isOutput=False)
    wv = nc.declare_dram_parameter("wv", [D, FKV], bf, isOutput=False)
    wo = nc.declare_dram_parameter("wo", [FQ, D], bf, isOutput=False)
    ccT = nc.declare_dram_parameter("ccT", [HD, S], bf, isOutput=False)
    ssT = nc.declare_dram_parameter("ssT", [HD, S], bf, isOutput=False)
    dmask = nc.declare_dram_parameter("dmask", [SC, 4 * QT], bf, isOutput=False)
    sel8 = nc.declare_dram_parameter("sel8", [SC, QH * QH], bf, isOutput=False)
    selbc = nc.declare_dram_parameter("selbc", [QH, QH * SC], bf, isOutput=False)
    out = nc.declare_dram_parameter("out", [S, D], bf, isOutput=True)

    hiT_r = hiT[:, :].rearrange("(c p) s -> p c s", p=128)
    wq_r = wq[:, :].rearrange("(c p) f -> p c f", p=128)
    wk_r = wk[:, :].rearrange("(c p) f -> p c f", p=128)
    wv_r = wv[:, :].rearrange("(c p) f -> p c f", p=128)
    wo_r = wo[:, :].rearrange("(h p) d -> p h d", p=128)

    with TileContext(nc) as tc, contextlib.ExitStack() as top:
        const_pool = top.enter_context(tc.tile_pool(name="const", bufs=1))
        persist = top.enter_context(tc.tile_pool(name="persist", bufs=1))
        psA = top.enter_context(tc.tile_pool(name="psA", bufs=4, space="PSUM"))
        psB = top.enter_context(tc.tile_pool(name="psB", bufs=2, space="PSUM"))

        # constants + small tables (DMA issue order = sync queue order;
        # projection weights first, rope/mask tables deferred until after
        # the first hidden-state slabs so the first K-pass starts ASAP)
        wkv_sb = const_pool.tile([128, 2, DMC, FKV], bf)
        nc.sync.dma_start(out=wkv_sb[:, 0], in_=wk_r)
        nc.sync.dma_start(out=wkv_sb[:, 1], in_=wv_r)
        cc_sb = const_pool.tile([HD, S], bf)
        ss_sb = const_pool.tile([HD, S], bf)
        dm_sb = const_pool.tile([SC, 4 * QT], bf)
        sel8_sb = const_pool.tile([SC, QH * QH], bf)
        selbc_sb = const_pool.tile([QH, QH * SC], bf)

        def _load_tables():
            nc.sync.dma_start(out=cc_sb, in_=ccT[:, :])
            nc.sync.dma_start(out=ss_sb, in_=ssT[:, :])
            nc.sync.dma_start(out=dm_sb, in_=dmask[:, :])
            nc.sync.dma_start(out=sel8_sb, in_=sel8[:, :])
            nc.sync.dma_start(out=selbc_sb, in_=selbc[:, :])

        ident = const_pool.tile([128, 128], bf)
        make_identity(nc, ident)

        # persistent activation stores
        ktr = persist.tile([128, KVH, S], bf)  # roped K^T  [d, kv, s]
        vt = persist.tile([128, KVH, NSC, HD], bf)  # V  [s-chunk, kv, sc, d]
        ctxr = persist.tile([128, QH, S], bf)  # ctx^T  [d, h, s]

        with contextlib.ExitStack() as proj_stack:
            slab_pool = proj_stack.enter_context(tc.tile_pool(name="slab", bufs=2))
            wqp_pool = proj_stack.enter_context(tc.tile_pool(name="wqp", bufs=2))
            stage_pool = proj_stack.enter_context(tc.tile_pool(name="stage", bufs=4))
            qcur_pool = proj_stack.enter_context(tc.tile_pool(name="qcur", bufs=2))
            pt_pool = proj_stack.enter_context(tc.tile_pool(name="pt", bufs=3))
            norm_pool = proj_stack.enter_context(tc.tile_pool(name="norm", bufs=2))
            qld_pool = proj_stack.enter_context(tc.tile_pool(name="qld", bufs=2))

            def rope_evac(pk, dest, ssl):
                """dest[:, :] = rope(pk) in bf16; dest is a [128, QT] AP.

                The partition-shifted multiplies read the PSUM accumulator
                directly: the walrus verifier only allows mismatched base
                partitions when the operands are in different memory spaces.
                """
                x = stage_pool.tile([128, QT], bf, name="ropex")
                nc.scalar.copy(x, pk)
                t1 = stage_pool.tile([128, QT], bf, name="ropet1")
                nc.vector.tensor_mul(t1, x, cc_sb[:, ssl])
                t2 = stage_pool.tile([128, QT], bf, name="ropet2")
                nc.vector.tensor_mul(t2[:64], pk[64:], ss_sb[:64, ssl])
                nc.vector.tensor_mul(t2[64:], pk[:64], ss_sb[64:, ssl])
                nc.vector.tensor_sub(dest[:64], t1[:64], t2[:64])
                nc.vector.tensor_add(dest[64:], t1[64:], t2[64:])

            for st in range(NQT):
                ssl = slice(st * QT, (st + 1) * QT)
                slabs = []
                for hh in range(2):
                    slab = slab_pool.tile([128, DMC // 2, QT], bf, name="slab")
                    nc.sync.dma_start(
                        out=slab,
                        in_=hiT_r[:, hh * (DMC // 2) : (hh + 1) * (DMC // 2), ssl],
                    )
                    slabs.append(slab)
                if st == 0:
                    _load_tables()

                qcur = qcur_pool.tile([128, QH, QT], bf, name="qcur")

                # 6 projection passes of 2 PSUM banks each:
                # (k0,k1), (v0,v1), (q0,q1), (q2,q3), (q4,q5), (q6,q7)
                passes = [("kv", 0), ("kv", 1)] + [("q", j) for j in range(4)]
                for kind, pj in passes:
                    if kind == "q":
                        wqp = wqp_pool.tile([128, DMC, 2 * HD], bf, name="wqp")
                        nc.sync.dma_start(
                            out=wqp, in_=wq_r[:, :, pj * 256 : (pj + 1) * 256]
                        )
                    banks = [
                        psA.tile([128, QT], f32, name="acc") for _ in range(2)
                    ]
                    for c in range(DMC):
                        mv = slabs[c // (DMC // 2)][:, c % (DMC // 2), :]
                        for j in range(2):
                            if kind == "kv":
                                stat = wkv_sb[:, pj, c, j * 128 : (j + 1) * 128]
                            else:
                                stat = wqp[:, c, j * 128 : (j + 1) * 128]
                            nc.tensor.matmul(
                                banks[j],
                                stat,
                                mv,
                                start=(c == 0),
                                stop=(c == DMC - 1),
                            )
                    if kind == "kv" and pj == 0:  # K heads
                        for j in range(2):
                            rope_evac(banks[j], ktr[:, j, ssl], ssl)
                    elif kind == "kv":  # V heads
                        for j in range(2):
                            v_sb = stage_pool.tile([128, QT], bf, name="vstage")
                            nc.scalar.copy(v_sb, banks[j])
                            for t in range(QT // 128):
                                ptr = psA.tile([128, 128], bf, name="acc")
                                nc.tensor.transpose(
                                    ptr, v_sb[:, t * 128 : (t + 1) * 128], ident
                                )
                                nc.vector.tensor_copy(
                                    vt[:, j, st * 4 + t, :], ptr
                                )
                    else:  # Q pair
                        for j in range(2):
                            rope_evac(banks[j], qcur[:, pj * 2 + j, :], ssl)

                # ---------------- attention for query tile st ----------------
                nk = G * (st + 1)
                npair = nk // 2
                pred = psA.tile([QH, QT], f32, name="acc")
                for h in range(QH):
                    kv = h // G
                    pctx = psA.tile([128, QT], f32, name="acc")
                    for p in range(npair):
                        pp = psB.tile([128, 2 * QT], f32, name="pair")
                        for half in range(2):
                            i = 2 * p + half
                            nc.tensor.matmul(
                                pp[:, half * QT : (half + 1) * QT],
                                ktr[:, kv, i * SC : (i + 1) * SC],
                                qcur[:, h, :],
                                start=True,
                                stop=True,
                            )
                        pt = pt_pool.tile([128, 2 * QT], bf, name="pt")
                        nc.scalar.activation(pt, pp, AF.Exp, scale=_SCALE)
                        if p >= npair - 2:  # diagonal pair -> 0/1 mask
                            t = p - (npair - 2)
                            nc.vector.tensor_mul(
                                pt, pt, dm_sb[:, t * 2 * QT : (t + 1) * 2 * QT]
                            )
                        for half in range(2):
                            i = 2 * p + half
                            ph = pt[:, half * QT : (half + 1) * QT]
                            nc.tensor.matmul(
                                pctx,
                                vt[:, kv, i, :],
                                ph,
                                start=(i == 0),
                                stop=(i == nk - 1),
                            )
                            nc.tensor.matmul(
                                pred,
                                sel8_sb[:, h * QH : (h + 1) * QH],
                                ph,
                                start=(h == 0 and i == 0),
                                stop=(h == QH - 1 and i == nk - 1),
                            )
                    # unnormalized ctx -> SBUF (normalized in place later)
                    nc.scalar.copy(ctxr[:, h, ssl], pctx)

                recip = norm_pool.tile([QH, QT], bf, name="recip")
                with nc.allow_low_precision(reason="bf16 softmax recip"):
                    nc.vector.reciprocal(recip, pred)
                for h in range(QH):
                    pbc = psA.tile([128, QT], f32, name="acc")
                    nc.tensor.matmul(
                        pbc,
                        selbc_sb[:, h * SC : (h + 1) * SC],
                        recip,
                        start=True,
                        stop=True,
                    )
                    nc.vector.tensor_mul(
                        ctxr[:, h, ssl], ctxr[:, h, ssl], pbc
                    )

        # ---------------- output projection (Wo streamed per d-tile) --------
        with contextlib.ExitStack() as d_stack:
            wod_pool = d_stack.enter_context(tc.tile_pool(name="wod", bufs=2))
            o_pool = d_stack.enter_context(tc.tile_pool(name="op", bufs=4))

            for dt in range(NDT):
                wod = wod_pool.tile([128, QH, DT], bf, name="wod")
                nc.sync.dma_start(out=wod, in_=wo_r[:, :, dt * DT : (dt + 1) * DT])
                for sc in range(NSC):
                    po = psA.tile([128, DT], f32, name="acc")
                    for h in range(QH):
                        nc.tensor.matmul(
                            po,
                            ctxr[:, h, sc * SC : (sc + 1) * SC],
                            wod[:, h, :],
                            start=(h == 0),
                            stop=(h == QH - 1),
                        )
                    o_sb = o_pool.tile([128, DT], bf, name="osb")
                    if sc % 2 == 0:
                        nc.scalar.copy(o_sb, po)
                    else:
                        nc.vector.tensor_copy(o_sb, po)
                    nc.sync.dma_start(
                        out=out[sc * SC : (sc + 1) * SC, dt * DT : (dt + 1) * DT],
                        in_=o_sb,
                    )

    _legalize_waits(nc)
    return nc


_NC_CACHE = {}
_last_exec_ns = None


def _get_nc():
    if "nc" not in _NC_CACHE:
        _NC_CACHE["nc"] = _build_nc()
    return _NC_CACHE["nc"]


# ---------------------------------------------------------------------------
# Optional NTFF profiling hook (used by the local test harness via
# KERNEL_TRACE=1; grading path leaves it off)
# ---------------------------------------------------------------------------
def _install_ntff_hook(so_path="/opt/axon/libaxon_pjrt.so"):
    if "antenv.axon_hooks" in sys.modules:
        return
    try:
        lib = ctypes.CDLL(so_path)
    except OSError:
        lib = None
    if lib is None or not hasattr(lib, "axon_start_nrt_profile"):
        hook = None
    else:
        lib.axon_start_nrt_profile.argtypes = [
            ctypes.POINTER(ctypes.c_int64),
            ctypes.c_size_t,
        ]
        lib.axon_start_nrt_profile.restype = ctypes.c_int64
        lib.axon_stop_nrt_profile.argtypes = [ctypes.c_char_p]
        lib.axon_stop_nrt_profile.restype = ctypes.c_int64

        @contextlib.contextmanager
        def hook(output_dir, device_ids):
            import jax

            jax.devices()
            if device_ids:
                ids = (ctypes.c_int64 * len(device_ids))(*device_ids)
                rc = lib.axon_start_nrt_profile(ids, len(device_ids))
            else:
                rc = lib.axon_start_nrt_profile(None, 0)
            if rc != 0:
                raise RuntimeError(f"axon_start_nrt_profile rc={rc}")
            try:
                yield
            finally:
                n = lib.axon_stop_nrt_profile(str(output_dir).encode())
                print(f"ntff profile: {n} file(s) -> {output_dir}", file=sys.stderr)

    mod = types.ModuleType("antenv.axon_hooks")
    mod.get_axon_ntff_profile_hook = lambda: hook
    sys.modules["antenv.axon_hooks"] = mod


# ---------------------------------------------------------------------------
# Host entry point
# ---------------------------------------------------------------------------
def kernel(hidden_states, position_ids, attention_mask, Wq, Wk, Wv, Wo):
    global _last_exec_ns
    from concourse import bass_utils

    hidden_states = np.asarray(hidden_states, dtype=np.float32)
    position_ids = np.asarray(position_ids)
    attention_mask = np.asarray(attention_mask)
    Wq = np.asarray(Wq, dtype=np.float32)
    Wk = np.asarray(Wk, dtype=np.float32)
    Wv = np.asarray(Wv, dtype=np.float32)
    Wo = np.asarray(Wo, dtype=np.float32)

    if not np.all(np.asarray(attention_mask) > 0):
        # Spec guarantees an all-ones mask; fall back to a host reference
        # implementation for the general case rather than mis-computing.
        return _host_reference(
            hidden_states, position_ids, attention_mask, Wq, Wk, Wv, Wo
        )

    # rope tables per batch: cc/ss [HD, S] with halves stacked
    half = HD // 2
    inv_freq = 1.0 / (THETA ** (np.arange(0, half, dtype=np.float32) / half))
    ccs, sss = [], []
    for b in range(B):
        freqs = position_ids[b].astype(np.float32)[:, None] * inv_freq[None, :]
        cosT = np.cos(freqs).T.astype(np.float32)  # [64, S]
        sinT = np.sin(freqs).T.astype(np.float32)
        ccs.append(
            np.ascontiguousarray(np.concatenate([cosT, cosT], axis=0).astype(BF16))
        )
        sss.append(
            np.ascontiguousarray(np.concatenate([sinT, sinT], axis=0).astype(BF16))
        )

    # diagonal-block 0/1 masks: block t: dmask[k, t*QT + q] = (q >= t*SC + k)
    kk = np.arange(SC)[:, None]
    qq = np.arange(QT)[None, :]
    dmask = np.concatenate(
        [(qq >= t * SC + kk).astype(np.float32) for t in range(G)], axis=1
    ).astype(BF16)
    dmask = np.ascontiguousarray(dmask)

    # head-selector stationaries for softmax row sums / broadcast
    sel8 = np.zeros((SC, QH * QH), dtype=np.float32)
    for h in range(QH):
        sel8[:, h * QH + h] = 1.0
    sel8 = np.ascontiguousarray(sel8.astype(BF16))
    selbc = np.zeros((QH, QH * SC), dtype=np.float32)
    for h in range(QH):
        selbc[h, h * SC : (h + 1) * SC] = 1.0
    selbc = np.ascontiguousarray(selbc.astype(BF16))

    hiTs = [np.ascontiguousarray(hidden_states[b].T.astype(BF16)) for b in range(B)]

    in_maps = []
    for c in range(NCORES):
        b = c // KV_SHARDS
        m = c % KV_SHARDS
        qcols = slice(m * FQ, (m + 1) * FQ)
        kvcols = slice(m * FKV, (m + 1) * FKV)
        in_maps.append(
            {
                "hiT": hiTs[b],
                "wq": np.ascontiguousarray(Wq[:, qcols].astype(BF16)),
                "wk": np.ascontiguousarray(Wk[:, kvcols].astype(BF16)),
                "wv": np.ascontiguousarray(Wv[:, kvcols].astype(BF16)),
                "wo": np.ascontiguousarray(Wo[qcols, :].astype(BF16)),
                "ccT": ccs[b],
                "ssT": sss[b],
                "dmask": dmask,
                "sel8": sel8,
                "selbc": selbc,
            }
        )

    nc = _get_nc()
    trace = os.environ.get("KERNEL_TRACE", "") == "1"
    if trace:
        _install_ntff_hook()
        bass_utils.upload_artifacts = lambda tmpdir: f"local:{tmpdir}"
    res = bass_utils.run_bass_kernel_spmd(
        nc, in_maps, list(range(NCORES)), trace=trace
    )
    _last_exec_ns = res.exec_time_ns

    out = np.zeros((B, S, D), dtype=np.float32)
    for c in range(NCORES):
        out[c // KV_SHARDS] += np.asarray(res.results[c]["out"]).astype(np.float32)
    return out


def _host_reference(hidden_states, position_ids, attention_mask, Wq, Wk, Wv, Wo):
    """Numpy fallback for inputs outside the spec's guarantees."""
    q = (hidden_states @ Wq).reshape(B, S, H, HD)
    k = (hidden_states @ Wk).reshape(B, S, HKV, HD)
    v = (hidden_states @ Wv).reshape(B, S, HKV, HD)

    half = HD // 2
    inv_freq = 1.0 / (THETA ** (np.arange(0, half, dtype=np.float32) / half))
    freqs = position_ids.astype(np.float32)[..., None] * inv_freq
    cos = np.cos(freqs)[:, :, None, :]
    sin = np.sin(freqs)[:, :, None, :]

    def rope(x):
        x1, x2 = x[..., :half], x[..., half:]
        return np.concatenate([x1 * cos - x2 * sin, x2 * cos + x1 * sin], axis=-1)

    q, k = rope(q), rope(k)
    qg = q.reshape(B, S, HKV, G, HD)
    scores = np.einsum("bqhgd,bkhd->bhgqk", qg, k) * (HD**-0.5)
    causal = np.tril(np.ones((S, S), bool))
    mask = causal[None, None, None] & (attention_mask[:, None, None, None, :] > 0)
    scores = np.where(mask, scores, np.finfo(np.float32).min)
    scores = scores - scores.max(axis=-1, keepdims=True)
    probs = np.exp(scores)
    probs = probs / probs.sum(axis=-1, keepdims=True)
    ctx = np.einsum("bhgqk,bkhd->bqhgd", probs, v).reshape(B, S, H * HD)
    return (ctx @ Wo).astype(np.float32)


# revision 14
# speedup vs baseline: 1.4661x; 1.0196x over previous
"""Trainium2 Bass kernel for nn_ExaoneAttention (dense transformer attention).

Full-input contract: kernel(**inputs) takes the unsharded inputs and returns
the full [B, S, D] output. Internally shards across 8 NeuronCores:
2-way data parallel over batch x 4-way tensor parallel over kv heads
(2 kv heads = 8 query heads per core). Each core computes a partial
output through its Wo row-slice; the host sums the 4 partials per batch.

v2 design (bf16 operands, f32 PSUM accumulation):
- Single fused pass: for each 512-token tile, project K/V/Q (weights
  stationary, hidden-state tile moving), rope on the fly, then run
  attention for that query tile against all keys so far. Output
  projection runs as a tail phase with Wo streamed per 512-column block.
  No DRAM staging round-trips; K/V/ctx live in SBUF for the whole kernel.
- Softmax in the "scoresT" orientation (keys on partitions, queries on
  the free dim). Per-chunk exp runs on 1024-wide PSUM pairs. Row sums
  accumulate via matmuls with a head-selector stationary into one
  [8, 512] PSUM tile per query tile, giving a single batched reciprocal;
  the reciprocal row is broadcast back over partitions with a tiny
  matmul and multiplied into the unnormalized context in place.
- Causal diagonal blocks are handled multiplicatively: exp first, then
  a 0/1 bf16 mask multiply (2x DVE throughput).
"""

import contextlib
import ctypes
import os
import sys
import types

import ml_dtypes
import numpy as np

# ---------------------------------------------------------------------------
# Problem constants (hardcoded per contract)
# ---------------------------------------------------------------------------
B, S, D = 2, 2048, 4096
H, HKV, HD = 32, 8, 128
G = H // HKV
THETA = 10000.0

NCORES = 8
BAT_SHARDS = 2
KV_SHARDS = 4
KVH = HKV // KV_SHARDS  # kv heads per core = 2
QH = KVH * G  # q heads per core = 8
FQ = QH * HD  # 1024
FKV = KVH * HD  # 256
DMC = D // 128  # 32 model-dim chunks

QT = 512  # query tile
NQT = S // QT  # 4
SC = 128  # sequence chunk
NSC = S // SC  # 16
DT = 512  # output d tile
NDT = D // DT  # 8

_SCALE = float(HD) ** -0.5
BF16 = ml_dtypes.bfloat16


# ---------------------------------------------------------------------------
# Wait-count legalization: this walrus build rejects instructions carrying
# more than a small number of sync waits (fused fp32/fp32r matmul: >1;
# drain: >4). Hoist excess waits onto standalone NoOps on the same engine
# immediately before the offending instruction; AND-semantics are preserved
# by sequential same-engine execution.
# ---------------------------------------------------------------------------
def _legalize_waits(nc):
    import bass_rust
    import concourse.mybir as mybir

    counter = 0
    for f in nc.m.functions:
        for bb in f.blocks:
            il = bb.instructions
            i = 0
            while i < len(il):
                ins = il[i]
                si = ins.sync_info
                if si is None or len(si.on_wait) <= 1:
                    i += 1
                    continue
                waits = list(si.on_wait)
                pos = i
                for w in waits[1:]:
                    counter += 1
                    nop = mybir.InstNoOp(name=f"lgw-{counter}", ins=[], outs=[])
                    nop.engine = ins.engine
                    nop.sync_info = bass_rust.SyncInfo(on_wait=[w], on_update=[])
                    il.insert(pos, nop)
                    pos += 1
                    i += 1
                ins.sync_info = bass_rust.SyncInfo(
                    on_wait=waits[:1], on_update=list(si.on_update)
                )
                i += 1
    return counter


# ---------------------------------------------------------------------------
# Bass kernel builder (per-core program; same program on all 8 cores)
# ---------------------------------------------------------------------------
def _build_nc():
    import concourse.bass as bass
    import concourse.mybir as mybir
    from concourse.masks import make_identity
    from concourse.tile import TileContext

    f32 = mybir.dt.float32
    bf = mybir.dt.bfloat16
    AF = mybir.ActivationFunctionType

    nc = bass.Bass()

    hiT = nc.declare_dram_parameter("hiT", [D, S], bf, isOutput=False)
    wq = nc.declare_dram_parameter("wq", [D, FQ], bf, isOutput=False)
    wk = nc.declare_dram_parameter("wk", [D, FKV], bf, isOutput=False)
    wv = nc.declare_dram_parameter("wv", [D, FKV], bf, isOutput=False)
    wo = nc.declare_dram_parameter("wo", [FQ, D], bf, isOutput=False)
    ccT = nc.declare_dram_parameter("ccT", [HD, S], bf, isOutput=False)
    ssT = nc.declare_dram_parameter("ssT", [HD, S], bf, isOutput=False)
    dmask = nc.declare_dram_parameter("dmask", [SC, 4 * QT], bf, isOutput=False)
    sel8 = nc.declare_dram_parameter("sel8", [SC, QH * QH], bf, isOutput=False)
    selbc = nc.declare_dram_parameter("selbc", [QH, QH * SC], bf, isOutput=False)
    out = nc.declare_dram_parameter("out", [S, D], bf, isOutput=True)

    hiT_r = hiT[:, :].rearrange("(c p) s -> p c s", p=128)
    wq_r = wq[:, :].rearrange("(c p) f -> p c f", p=128)
    wk_r = wk[:, :].rearrange("(c p) f -> p c f", p=128)
    wv_r = wv[:, :].rearrange("(c p) f -> p c f", p=128)
    wo_r = wo[:, :].rearrange("(h p) d -> p h d", p=128)

    with TileContext(nc) as tc, contextlib.ExitStack() as top:
        const_pool = top.enter_context(tc.tile_pool(name="const", bufs=1))
        persist = top.enter_context(tc.tile_pool(name="persist", bufs=1))
        psA = top.enter_context(tc.tile_pool(name="psA", bufs=4, space="PSUM"))
        psB = top.enter_context(tc.tile_pool(name="psB", bufs=2, space="PSUM"))

        # constants + small tables: issued on the Activation HWDGE queue so
        # they transfer in parallel with the hidden-state slabs on the sync
        # queue (two independent hardware DGE rings).
        wkv_sb = const_pool.tile([128, 2, DMC, FKV], bf)
        nc.scalar.dma_start(out=wkv_sb[:, 0], in_=wk_r)
        cc_sb = const_pool.tile([HD, S], bf)
        nc.scalar.dma_start(out=cc_sb, in_=ccT[:, :])
        ss_sb = const_pool.tile([HD, S], bf)
        nc.scalar.dma_start(out=ss_sb, in_=ssT[:, :])
        nc.scalar.dma_start(out=wkv_sb[:, 1], in_=wv_r)
        dm_sb = const_pool.tile([SC, 4 * QT], bf)
        nc.scalar.dma_start(out=dm_sb, in_=dmask[:, :])
        sel8_sb = const_pool.tile([SC, QH * QH], bf)
        nc.scalar.dma_start(out=sel8_sb, in_=sel8[:, :])
        selbc_sb = const_pool.tile([QH, QH * SC], bf)
        nc.scalar.dma_start(out=selbc_sb, in_=selbc[:, :])

        ident = const_pool.tile([128, 128], bf)
        make_identity(nc, ident)

        # persistent activation stores
        ktr = persist.tile([128, KVH, S], bf)  # roped K^T  [d, kv, s]
        vt = persist.tile([128, KVH, NSC, HD], bf)  # V  [s-chunk, kv, sc, d]
        ctxr = persist.tile([128, QH, S], bf)  # ctx^T  [d, h, s]

        with contextlib.ExitStack() as proj_stack:
            slab_pool = proj_stack.enter_context(tc.tile_pool(name="slab", bufs=4))
            wqp_pool = proj_stack.enter_context(tc.tile_pool(name="wqp", bufs=2))
            stage_pool = proj_stack.enter_context(tc.tile_pool(name="stage", bufs=4))
            qcur_pool = proj_stack.enter_context(tc.tile_pool(name="qcur", bufs=2))
            pt_pool = proj_stack.enter_context(tc.tile_pool(name="pt", bufs=3))
            norm_pool = proj_stack.enter_context(tc.tile_pool(name="norm", bufs=2))
            qld_pool = proj_stack.enter_context(tc.tile_pool(name="qld", bufs=2))

            def rope_evac(pk, dest, ssl):
                """dest[:, :] = rope(pk) in bf16; dest is a [128, QT] AP.

                The partition-shifted multiplies read the PSUM accumulator
                directly: the walrus verifier only allows mismatched base
                partitions when the operands are in different memory spaces.
                """
                x = stage_pool.tile([128, QT], bf, name="ropex")
                nc.scalar.copy(x, pk)
                t1 = stage_pool.tile([128, QT], bf, name="ropet1")
                nc.vector.tensor_mul(t1, x, cc_sb[:, ssl])
                t2 = stage_pool.tile([128, QT], bf, name="ropet2")
                nc.vector.tensor_mul(t2[:64], pk[64:], ss_sb[:64, ssl])
                nc.vector.tensor_mul(t2[64:], pk[:64], ss_sb[64:, ssl])
                nc.vector.tensor_sub(dest[:64], t1[:64], t2[:64])
                nc.vector.tensor_add(dest[64:], t1[64:], t2[64:])

            def normalize(pred_sb, ssl_p):
                """probs denominators -> 1/sum, broadcast, scale ctx in place."""
                recip = norm_pool.tile([QH, QT], bf, name="recip")
                with nc.allow_low_precision(reason="bf16 softmax recip"):
                    nc.vector.reciprocal(recip, pred_sb)
                for h in range(QH):
                    pbc = psA.tile([128, QT], f32, name="acc")
                    nc.tensor.matmul(
                        pbc,
                        selbc_sb[:, h * SC : (h + 1) * SC],
                        recip,
                        start=True,
                        stop=True,
                    )
                    nc.vector.tensor_mul(
                        ctxr[:, h, ssl_p], ctxr[:, h, ssl_p], pbc
                    )

            pending = None  # deferred (pred_sb, ssl) from the previous tile
            NSLAB = 4
            for st in range(NQT):
                ssl = slice(st * QT, (st + 1) * QT)
                slabs = []
                for hh in range(NSLAB):
                    slab = slab_pool.tile([128, DMC // NSLAB, QT], bf, name="slab")
                    nc.sync.dma_start(
                        out=slab,
                        in_=hiT_r[
                            :, hh * (DMC // NSLAB) : (hh + 1) * (DMC // NSLAB), ssl
                        ],
                    )
                    slabs.append(slab)

                qcur = qcur_pool.tile([128, QH, QT], bf, name="qcur")

                # 6 projection passes of 2 PSUM banks each:
                # (k0,k1), (v0,v1), (q0,q1), (q2,q3), (q4,q5), (q6,q7)
                passes = [("kv", 0), ("kv", 1)] + [("q", j) for j in range(4)]
                for kind, pj in passes:
                    if kind == "q":
                        wqp = wqp_pool.tile([128, DMC, 2 * HD], bf, name="wqp")
                        nc.sync.dma_start(
                            out=wqp, in_=wq_r[:, :, pj * 256 : (pj + 1) * 256]
                        )
                    banks = [
                        psA.tile([128, QT], f32, name="acc") for _ in range(2)
                    ]
                    for c in range(DMC):
                        mv = slabs[c // (DMC // NSLAB)][:, c % (DMC // NSLAB), :]
                        for j in range(2):
                            if kind == "kv":
                                stat = wkv_sb[:, pj, c, j * 128 : (j + 1) * 128]
                            else:
                                stat = wqp[:, c, j * 128 : (j + 1) * 128]
                            nc.tensor.matmul(
                                banks[j],
                                stat,
                                mv,
                                start=(c == 0),
                                stop=(c == DMC - 1),
                            )
                    if kind == "kv" and pj == 0:  # K heads
                        for j in range(2):
                            rope_evac(banks[j], ktr[:, j, ssl], ssl)
                    elif kind == "kv":  # V heads
                        for j in range(2):
                            v_sb = stage_pool.tile([128, QT], bf, name="vstage")
                            nc.scalar.copy(v_sb, banks[j])
                            for t in range(QT // 128):
                                ptr = psA.tile([128, 128], bf, name="acc")
                                nc.tensor.transpose(
                                    ptr, v_sb[:, t * 128 : (t + 1) * 128], ident
                                )
                                nc.vector.tensor_copy(
                                    vt[:, j, st * 4 + t, :], ptr
                                )
                    else:  # Q pair
                        for j in range(2):
                            rope_evac(banks[j], qcur[:, pj * 2 + j, :], ssl)

                # previous tile's softmax normalization: emitted here so its
                # PE work (broadcast matmuls) sits behind this tile's
                # projection matmuls, hiding the DVE reciprocal latency
                if pending is not None:
                    normalize(*pending)
                    pending = None

                # ---------------- attention for query tile st ----------------
                nk = G * (st + 1)
                npair = nk // 2
                pred = psA.tile([QH, QT], f32, name="acc")
                for h in range(QH):
                    kv = h // G
                    pctx = psA.tile([128, QT], f32, name="acc")
                    for p in range(npair):
                        pp = psB.tile([128, 2 * QT], f32, name="pair")
                        for half in range(2):
                            i = 2 * p + half
                            nc.tensor.matmul(
                                pp[:, half * QT : (half + 1) * QT],
                                ktr[:, kv, i * SC : (i + 1) * SC],
                                qcur[:, h, :],
                                start=True,
                                stop=True,
                            )
                        pt = pt_pool.tile([128, 2 * QT], bf, name="pt")
                        nc.scalar.activation(pt, pp, AF.Exp, scale=_SCALE)
                        if p >= npair - 2:  # diagonal pair -> 0/1 mask
                            t = p - (npair - 2)
                            nc.vector.tensor_mul(
                                pt, pt, dm_sb[:, t * 2 * QT : (t + 1) * 2 * QT]
                            )
                        for half in range(2):
                            i = 2 * p + half
                            ph = pt[:, half * QT : (half + 1) * QT]
                            nc.tensor.matmul(
                                pctx,
                                vt[:, kv, i, :],
                                ph,
                                start=(i == 0),
                                stop=(i == nk - 1),
                            )
                            nc.tensor.matmul(
                                pred,
                                sel8_sb[:, h * QH : (h + 1) * QH],
                                ph,
                                start=(h == 0 and i == 0),
                                stop=(h == QH - 1 and i == nk - 1),
                            )
                    # unnormalized ctx -> SBUF (normalized in place later)
                    nc.scalar.copy(ctxr[:, h, ssl], pctx)

                # free the PSUM bank; the reciprocal + broadcast run after the
                # next tile's projection passes
                pred_sb = norm_pool.tile([QH, QT], f32, name="predsb")
                nc.scalar.copy(pred_sb, pred)
                pending = (pred_sb, ssl)

            normalize(*pending)

        # ---------------- output projection (Wo streamed per d-tile) --------
        with contextlib.ExitStack() as d_stack:
            wod_pool = d_stack.enter_context(tc.tile_pool(name="wod", bufs=2))
            o_pool = d_stack.enter_context(tc.tile_pool(name="op", bufs=4))

            for dt in range(NDT):
                wod = wod_pool.tile([128, QH, DT], bf, name="wod")
                nc.sync.dma_start(out=wod, in_=wo_r[:, :, dt * DT : (dt + 1) * DT])
                for sc in range(NSC):
                    po = psA.tile([128, DT], f32, name="acc")
                    for h in range(QH):
                        nc.tensor.matmul(
                            po,
                            ctxr[:, h, sc * SC : (sc + 1) * SC],
                            wod[:, h, :],
                            start=(h == 0),
                            stop=(h == QH - 1),
                        )
                    o_sb = o_pool.tile([128, DT], bf, name="osb")
                    if sc % 2 == 0:
                        nc.scalar.copy(o_sb, po)
                    else:
                        nc.vector.tensor_copy(o_sb, po)
                    nc.sync.dma_start(
                        out=out[sc * SC : (sc + 1) * SC, dt * DT : (dt + 1) * DT],
                        in_=o_sb,
                    )

    _legalize_waits(nc)
    return nc


_NC_CACHE = {}
_last_exec_ns = None


def _get_nc():
    if "nc" not in _NC_CACHE:
        _NC_CACHE["nc"] = _build_nc()
    return _NC_CACHE["nc"]


# ---------------------------------------------------------------------------
# Optional NTFF profiling hook (used by the local test harness via
# KERNEL_TRACE=1; grading path leaves it off)
# ---------------------------------------------------------------------------
def _install_ntff_hook(so_path="/opt/axon/libaxon_pjrt.so"):
    if "antenv.axon_hooks" in sys.modules:
        return
    try:
        lib = ctypes.CDLL(so_path)
    except OSError:
        lib = None
    if lib is None or not hasattr(lib, "axon_start_nrt_profile"):
        hook = None
    else:
        lib.axon_start_nrt_profile.argtypes = [
            ctypes.POINTER(ctypes.c_int64),
            ctypes.c_size_t,
        ]
        lib.axon_start_nrt_profile.restype = ctypes.c_int64
        lib.axon_stop_nrt_profile.argtypes = [ctypes.c_char_p]
        lib.axon_stop_nrt_profile.restype = ctypes.c_int64

        @contextlib.contextmanager
        def hook(output_dir, device_ids):
            import jax

            jax.devices()
            if device_ids:
                ids = (ctypes.c_int64 * len(device_ids))(*device_ids)
                rc = lib.axon_start_nrt_profile(ids, len(device_ids))
            else:
                rc = lib.axon_start_nrt_profile(None, 0)
            if rc != 0:
                raise RuntimeError(f"axon_start_nrt_profile rc={rc}")
            try:
                yield
            finally:
                n = lib.axon_stop_nrt_profile(str(output_dir).encode())
                print(f"ntff profile: {n} file(s) -> {output_dir}", file=sys.stderr)

    mod = types.ModuleType("antenv.axon_hooks")
    mod.get_axon_ntff_profile_hook = lambda: hook
    sys.modules["antenv.axon_hooks"] = mod


# ---------------------------------------------------------------------------
# Host entry point
# ---------------------------------------------------------------------------
def kernel(hidden_states, position_ids, attention_mask, Wq, Wk, Wv, Wo):
    global _last_exec_ns
    from concourse import bass_utils

    hidden_states = np.asarray(hidden_states, dtype=np.float32)
    position_ids = np.asarray(position_ids)
    attention_mask = np.asarray(attention_mask)
    Wq = np.asarray(Wq, dtype=np.float32)
    Wk = np.asarray(Wk, dtype=np.float32)
    Wv = np.asarray(Wv, dtype=np.float32)
    Wo = np.asarray(Wo, dtype=np.float32)

    if not np.all(np.asarray(attention_mask) > 0):
        # Spec guarantees an all-ones mask; fall back to a host reference
        # implementation for the general case rather than mis-computing.
        return _host_reference(
            hidden_states, position_ids, attention_mask, Wq, Wk, Wv, Wo
        )

    # rope tables per batch: cc/ss [HD, S] with halves stacked
    half = HD // 2
    inv_freq = 1.0 / (THETA ** (np.arange(0, half, dtype=np.float32) / half))
    ccs, sss = [], []
    for b in range(B):
        freqs = position_ids[b].astype(np.float32)[:, None] * inv_freq[None, :]
        cosT = np.cos(freqs).T.astype(np.float32)  # [64, S]
        sinT = np.sin(freqs).T.astype(np.float32)
        ccs.append(
            np.ascontiguousarray(np.concatenate([cosT, cosT], axis=0).astype(BF16))
        )
        sss.append(
            np.ascontiguousarray(np.concatenate([sinT, sinT], axis=0).astype(BF16))
        )

    # diagonal-block 0/1 masks: block t: dmask[k, t*QT + q] = (q >= t*SC + k)
    kk = np.arange(SC)[:, None]
    qq = np.arange(QT)[None, :]
    dmask = np.concatenate(
        [(qq >= t * SC + kk).astype(np.float32) for t in range(G)], axis=1
    ).astype(BF16)
    dmask = np.ascontiguousarray(dmask)

    # head-selector stationaries for softmax row sums / broadcast
    sel8 = np.zeros((SC, QH * QH), dtype=np.float32)
    for h in range(QH):
        sel8[:, h * QH + h] = 1.0
    sel8 = np.ascontiguousarray(sel8.astype(BF16))
    selbc = np.zeros((QH, QH * SC), dtype=np.float32)
    for h in range(QH):
        selbc[h, h * SC : (h + 1) * SC] = 1.0
    selbc = np.ascontiguousarray(selbc.astype(BF16))

    hiTs = [np.ascontiguousarray(hidden_states[b].T.astype(BF16)) for b in range(B)]

    in_maps = []
    for c in range(NCORES):
        b = c // KV_SHARDS
        m = c % KV_SHARDS
        qcols = slice(m * FQ, (m + 1) * FQ)
        kvcols = slice(m * FKV, (m + 1) * FKV)
        in_maps.append(
            {
                "hiT": hiTs[b],
                "wq": np.ascontiguousarray(Wq[:, qcols].astype(BF16)),
                "wk": np.ascontiguousarray(Wk[:, kvcols].astype(BF16)),
                "wv": np.ascontiguousarray(Wv[:, kvcols].astype(BF16)),
                "wo": np.ascontiguousarray(Wo[qcols, :].astype(BF16)),
                "ccT": ccs[b],
                "ssT": sss[b],
                "dmask": dmask,
                "sel8": sel8,
                "selbc": selbc,
            }
        )

    nc = _get_nc()
    trace = os.environ.get("KERNEL_TRACE", "") == "1"
    if trace:
        _install_ntff_hook()
        bass_utils.upload_artifacts = lambda tmpdir: f"local:{tmpdir}"
    res = bass_utils.run_bass_kernel_spmd(
        nc, in_maps, list(range(NCORES)), trace=trace
    )
    _last_exec_ns = res.exec_time_ns

    out = np.zeros((B, S, D), dtype=np.float32)
    for c in range(NCORES):
        out[c // KV_SHARDS] += np.asarray(res.results[c]["out"]).astype(np.float32)
    return out


def _host_reference(hidden_states, position_ids, attention_mask, Wq, Wk, Wv, Wo):
    """Numpy fallback for inputs outside the spec's guarantees."""
    q = (hidden_states @ Wq).reshape(B, S, H, HD)
    k = (hidden_states @ Wk).reshape(B, S, HKV, HD)
    v = (hidden_states @ Wv).reshape(B, S, HKV, HD)

    half = HD // 2
    inv_freq = 1.0 / (THETA ** (np.arange(0, half, dtype=np.float32) / half))
    freqs = position_ids.astype(np.float32)[..., None] * inv_freq
    cos = np.cos(freqs)[:, :, None, :]
    sin = np.sin(freqs)[:, :, None, :]

    def rope(x):
        x1, x2 = x[..., :half], x[..., half:]
        return np.concatenate([x1 * cos - x2 * sin, x2 * cos + x1 * sin], axis=-1)

    q, k = rope(q), rope(k)
    qg = q.reshape(B, S, HKV, G, HD)
    scores = np.einsum("bqhgd,bkhd->bhgqk", qg, k) * (HD**-0.5)
    causal = np.tril(np.ones((S, S), bool))
    mask = causal[None, None, None] & (attention_mask[:, None, None, None, :] > 0)
    scores = np.where(mask, scores, np.finfo(np.float32).min)
    scores = scores - scores.max(axis=-1, keepdims=True)
    probs = np.exp(scores)
    probs = probs / probs.sum(axis=-1, keepdims=True)
    ctx = np.einsum("bhgqk,bkhd->bqhgd", probs, v).reshape(B, S, H * HD)
    return (ctx @ Wo).astype(np.float32)


# revision 18
# speedup vs baseline: 1.4942x; 1.0191x over previous
"""Trainium2 Bass kernel for nn_ExaoneAttention (dense transformer attention).

Full-input contract: kernel(**inputs) takes the unsharded inputs and returns
the full [B, S, D] output. Internally shards across 8 NeuronCores:
2-way data parallel over batch x 4-way tensor parallel over kv heads
(2 kv heads = 8 query heads per core). Each core computes a partial
output through its Wo row-slice; the host sums the 4 partials per batch.

v2 design (bf16 operands, f32 PSUM accumulation):
- Single fused pass: for each 512-token tile, project K/V/Q (weights
  stationary, hidden-state tile moving), rope on the fly, then run
  attention for that query tile against all keys so far. Output
  projection runs as a tail phase with Wo streamed per 512-column block.
  No DRAM staging round-trips; K/V/ctx live in SBUF for the whole kernel.
- Softmax in the "scoresT" orientation (keys on partitions, queries on
  the free dim). Per-chunk exp runs on 1024-wide PSUM pairs. Row sums
  accumulate via matmuls with a head-selector stationary into one
  [8, 512] PSUM tile per query tile, giving a single batched reciprocal;
  the reciprocal row is broadcast back over partitions with a tiny
  matmul and multiplied into the unnormalized context in place.
- Causal diagonal blocks are handled multiplicatively: exp first, then
  a 0/1 bf16 mask multiply (2x DVE throughput).
"""

import contextlib
import ctypes
import os
import sys
import types

import ml_dtypes
import numpy as np

# ---------------------------------------------------------------------------
# Problem constants (hardcoded per contract)
# ---------------------------------------------------------------------------
B, S, D = 2, 2048, 4096
H, HKV, HD = 32, 8, 128
G = H // HKV
THETA = 10000.0

NCORES = 8
BAT_SHARDS = 2
KV_SHARDS = 4
KVH = HKV // KV_SHARDS  # kv heads per core = 2
QH = KVH * G  # q heads per core = 8
FQ = QH * HD  # 1024
FKV = KVH * HD  # 256
DMC = D // 128  # 32 model-dim chunks

QT = 512  # query tile
NQT = S // QT  # 4
SC = 128  # sequence chunk
NSC = S // SC  # 16
DT = 512  # output d tile
NDT = D // DT  # 8

_SCALE = float(HD) ** -0.5
BF16 = ml_dtypes.bfloat16


# ---------------------------------------------------------------------------
# Wait-count legalization: this walrus build rejects instructions carrying
# more than a small number of sync waits (fused fp32/fp32r matmul: >1;
# drain: >4). Hoist excess waits onto standalone NoOps on the same engine
# immediately before the offending instruction; AND-semantics are preserved
# by sequential same-engine execution.
# ---------------------------------------------------------------------------
def _legalize_waits(nc):
    import bass_rust
    import concourse.mybir as mybir

    counter = 0
    for f in nc.m.functions:
        for bb in f.blocks:
            il = bb.instructions
            i = 0
            while i < len(il):
                ins = il[i]
                si = ins.sync_info
                if si is None or len(si.on_wait) <= 1:
                    i += 1
                    continue
                waits = list(si.on_wait)
                pos = i
                for w in waits[1:]:
                    counter += 1
                    nop = mybir.InstNoOp(name=f"lgw-{counter}", ins=[], outs=[])
                    nop.engine = ins.engine
                    nop.sync_info = bass_rust.SyncInfo(on_wait=[w], on_update=[])
                    il.insert(pos, nop)
                    pos += 1
                    i += 1
                ins.sync_info = bass_rust.SyncInfo(
                    on_wait=waits[:1], on_update=list(si.on_update)
                )
                i += 1
    return counter


# ---------------------------------------------------------------------------
# Bass kernel builder (per-core program; same program on all 8 cores)
# ---------------------------------------------------------------------------
def _build_nc():
    import concourse.bass as bass
    import concourse.mybir as mybir
    from concourse.masks import make_identity
    from concourse.tile import TileContext

    f32 = mybir.dt.float32
    bf = mybir.dt.bfloat16
    AF = mybir.ActivationFunctionType

    nc = bass.Bass()

    hiT = nc.declare_dram_parameter("hiT", [D, S], bf, isOutput=False)
    wq = nc.declare_dram_parameter("wq", [D, FQ], bf, isOutput=False)
    wk = nc.declare_dram_parameter("wk", [D, FKV], bf, isOutput=False)
    wv = nc.declare_dram_parameter("wv", [D, FKV], bf, isOutput=False)
    wo = nc.declare_dram_parameter("wo", [FQ, D], bf, isOutput=False)
    ccT = nc.declare_dram_parameter("ccT", [HD, S], bf, isOutput=False)
    ssT = nc.declare_dram_parameter("ssT", [HD, S], bf, isOutput=False)
    dmask = nc.declare_dram_parameter("dmask", [SC, 4 * QT], bf, isOutput=False)
    sel8 = nc.declare_dram_parameter("sel8", [SC, QH * QH], bf, isOutput=False)
    selbc = nc.declare_dram_parameter("selbc", [QH, QH * SC], bf, isOutput=False)
    out = nc.declare_dram_parameter("out", [S, D], bf, isOutput=True)

    hiT_r = hiT[:, :].rearrange("(c p) s -> p c s", p=128)
    wq_r = wq[:, :].rearrange("(c p) f -> p c f", p=128)
    wk_r = wk[:, :].rearrange("(c p) f -> p c f", p=128)
    wv_r = wv[:, :].rearrange("(c p) f -> p c f", p=128)
    wo_r = wo[:, :].rearrange("(h p) d -> p h d", p=128)

    with TileContext(nc) as tc, contextlib.ExitStack() as top:
        const_pool = top.enter_context(tc.tile_pool(name="const", bufs=1))
        persist = top.enter_context(tc.tile_pool(name="persist", bufs=1))
        psA = top.enter_context(tc.tile_pool(name="psA", bufs=4, space="PSUM"))
        psB = top.enter_context(tc.tile_pool(name="psB", bufs=2, space="PSUM"))

        # constants + small tables: issued on the Activation HWDGE queue so
        # they transfer in parallel with the hidden-state slabs on the sync
        # queue (two independent hardware DGE rings).
        wk_sb = const_pool.tile([128, DMC, FKV], bf)
        nc.scalar.dma_start(out=wk_sb, in_=wk_r)
        wv_sb = const_pool.tile([128, DMC, FKV], bf)
        nc.scalar.dma_start(out=wv_sb, in_=wv_r)
        cc_sb = const_pool.tile([HD, S], bf)
        nc.scalar.dma_start(out=cc_sb, in_=ccT[:, :])
        ss_sb = const_pool.tile([HD, S], bf)
        nc.scalar.dma_start(out=ss_sb, in_=ssT[:, :])
        dm_sb = const_pool.tile([SC, 4 * QT], bf)
        nc.scalar.dma_start(out=dm_sb, in_=dmask[:, :])
        sel8_sb = const_pool.tile([SC, QH * QH], bf)
        nc.scalar.dma_start(out=sel8_sb, in_=sel8[:, :])
        selbc_sb = const_pool.tile([QH, QH * SC], bf)
        nc.scalar.dma_start(out=selbc_sb, in_=selbc[:, :])

        ident = const_pool.tile([128, 128], bf)
        make_identity(nc, ident)

        # persistent activation stores
        ktr = persist.tile([128, KVH, S], bf)  # roped K^T  [d, kv, s]
        vt = persist.tile([128, KVH, NSC, HD], bf)  # V  [s-chunk, kv, sc, d]
        ctxr = persist.tile([128, QH, S], bf)  # ctx^T  [d, h, s]

        with contextlib.ExitStack() as proj_stack:
            slab_pool = proj_stack.enter_context(tc.tile_pool(name="slab", bufs=4))
            wqp_pool = proj_stack.enter_context(tc.tile_pool(name="wqp", bufs=2))
            stage_pool = proj_stack.enter_context(tc.tile_pool(name="stage", bufs=4))
            qcur_pool = proj_stack.enter_context(tc.tile_pool(name="qcur", bufs=2))
            pt_pool = proj_stack.enter_context(tc.tile_pool(name="pt", bufs=3))
            norm_pool = proj_stack.enter_context(tc.tile_pool(name="norm", bufs=2))
            qld_pool = proj_stack.enter_context(tc.tile_pool(name="qld", bufs=2))

            def rope_evac(pk, dest, ssl):
                """dest[:, :] = rope(pk) in bf16; dest is a [128, QT] AP.

                The partition-shifted multiplies read the PSUM accumulator
                directly: the walrus verifier only allows mismatched base
                partitions when the operands are in different memory spaces.
                """
                x = stage_pool.tile([128, QT], bf, name="ropex")
                nc.scalar.copy(x, pk)
                t1 = stage_pool.tile([128, QT], bf, name="ropet1")
                nc.vector.tensor_mul(t1, x, cc_sb[:, ssl])
                t2 = stage_pool.tile([128, QT], bf, name="ropet2")
                nc.vector.tensor_mul(t2[:64], pk[64:], ss_sb[:64, ssl])
                nc.vector.tensor_mul(t2[64:], pk[:64], ss_sb[64:, ssl])
                nc.vector.tensor_sub(dest[:64], t1[:64], t2[:64])
                nc.vector.tensor_add(dest[64:], t1[64:], t2[64:])

            def normalize(pred_sb, ssl_p):
                """probs denominators -> 1/sum, broadcast, scale ctx in place."""
                recip = norm_pool.tile([QH, QT], bf, name="recip")
                with nc.allow_low_precision(reason="bf16 softmax recip"):
                    nc.vector.reciprocal(recip, pred_sb)
                for h in range(QH):
                    pbc = psA.tile([128, QT], f32, name="acc")
                    nc.tensor.matmul(
                        pbc,
                        selbc_sb[:, h * SC : (h + 1) * SC],
                        recip,
                        start=True,
                        stop=True,
                    )
                    nc.vector.tensor_mul(
                        ctxr[:, h, ssl_p], ctxr[:, h, ssl_p], pbc
                    )

            pending = None  # deferred (pred_sb, ssl) from the previous tile
            NSLAB = 4
            for st in range(NQT):
                ssl = slice(st * QT, (st + 1) * QT)
                slabs = []
                for hh in range(NSLAB):
                    slab = slab_pool.tile([128, DMC // NSLAB, QT], bf, name="slab")
                    nc.sync.dma_start(
                        out=slab,
                        in_=hiT_r[
                            :, hh * (DMC // NSLAB) : (hh + 1) * (DMC // NSLAB), ssl
                        ],
                    )
                    slabs.append(slab)

                qcur = qcur_pool.tile([128, QH, QT], bf, name="qcur")

                # 6 projection passes of 2 PSUM banks each:
                # (k0,k1), (v0,v1), (q0,q1), (q2,q3), (q4,q5), (q6,q7)
                passes = [("kv", 0), ("kv", 1)] + [("q", j) for j in range(4)]
                for kind, pj in passes:
                    if kind == "q":
                        wqp = wqp_pool.tile([128, DMC, 2 * HD], bf, name="wqp")
                        nc.sync.dma_start(
                            out=wqp, in_=wq_r[:, :, pj * 256 : (pj + 1) * 256]
                        )
                    banks = [
                        psA.tile([128, QT], f32, name="acc") for _ in range(2)
                    ]
                    for c in range(DMC):
                        mv = slabs[c // (DMC // NSLAB)][:, c % (DMC // NSLAB), :]
                        for j in range(2):
                            if kind == "kv":
                                wsrc = wk_sb if pj == 0 else wv_sb
                                stat = wsrc[:, c, j * 128 : (j + 1) * 128]
                            else:
                                stat = wqp[:, c, j * 128 : (j + 1) * 128]
                            nc.tensor.matmul(
                                banks[j],
                                stat,
                                mv,
                                start=(c == 0),
                                stop=(c == DMC - 1),
                            )
                    if kind == "kv" and pj == 0:  # K heads
                        for j in range(2):
                            rope_evac(banks[j], ktr[:, j, ssl], ssl)
                    elif kind == "kv":  # V heads
                        for j in range(2):
                            v_sb = stage_pool.tile([128, QT], bf, name="vstage")
                            nc.scalar.copy(v_sb, banks[j])
                            for t in range(QT // 128):
                                ptr = psA.tile([128, 128], bf, name="acc")
                                nc.tensor.transpose(
                                    ptr, v_sb[:, t * 128 : (t + 1) * 128], ident
                                )
                                nc.vector.tensor_copy(
                                    vt[:, j, st * 4 + t, :], ptr
                                )
                    else:  # Q pair
                        for j in range(2):
                            rope_evac(banks[j], qcur[:, pj * 2 + j, :], ssl)

                # previous tile's softmax normalization: emitted here so its
                # PE work (broadcast matmuls) sits behind this tile's
                # projection matmuls, hiding the DVE reciprocal latency
                if pending is not None:
                    normalize(*pending)
                    pending = None

                # ---------------- attention for query tile st ----------------
                nk = G * (st + 1)
                npair = nk // 2
                pred = psA.tile([QH, QT], f32, name="acc")
                for h in range(QH):
                    kv = h // G
                    pctx = psA.tile([128, QT], f32, name="acc")

                    def flush_pv(pt_p):
                        """PV + row-sum matmuls for a softmaxed pair."""
                        pt_f, p_f = pt_p
                        for half in range(2):
                            i = 2 * p_f + half
                            ph = pt_f[:, half * QT : (half + 1) * QT]
                            nc.tensor.matmul(
                                pctx,
                                vt[:, kv, i, :],
                                ph,
                                start=(i == 0),
                                stop=(i == nk - 1),
                            )
                            nc.tensor.matmul(
                                pred,
                                sel8_sb[:, h * QH : (h + 1) * QH],
                                ph,
                                start=(h == 0 and i == 0),
                                stop=(h == QH - 1 and i == nk - 1),
                            )

                    # software-pipelined with lag 2: the PV matmuls for pair p
                    # are emitted after the scores of pair p+2, so the PE never
                    # waits on the ACT exp of the pair it is about to consume
                    ptq = []
                    for p in range(npair):
                        pp = psB.tile([128, 2 * QT], f32, name="pair")
                        for half in range(2):
                            i = 2 * p + half
                            nc.tensor.matmul(
                                pp[:, half * QT : (half + 1) * QT],
                                ktr[:, kv, i * SC : (i + 1) * SC],
                                qcur[:, h, :],
                                start=True,
                                stop=True,
                            )
                        pt = pt_pool.tile([128, 2 * QT], bf, name="pt")
                        nc.scalar.activation(pt, pp, AF.Exp, scale=_SCALE)
                        if p >= npair - 2:  # diagonal pair -> 0/1 mask
                            t = p - (npair - 2)
                            nc.vector.tensor_mul(
                                pt, pt, dm_sb[:, t * 2 * QT : (t + 1) * 2 * QT]
                            )
                        ptq.append((pt, p))
                        if len(ptq) > 2:
                            flush_pv(ptq.pop(0))
                    for pt_p in ptq:
                        flush_pv(pt_p)
                    # unnormalized ctx -> SBUF (normalized in place later)
                    nc.scalar.copy(ctxr[:, h, ssl], pctx)

                # free the PSUM bank; the reciprocal + broadcast run after the
                # next tile's projection passes
                pred_sb = norm_pool.tile([QH, QT], f32, name="predsb")
                nc.scalar.copy(pred_sb, pred)
                pending = (pred_sb, ssl)

            normalize(*pending)

        # ---------------- output projection (Wo streamed per d-tile) --------
        with contextlib.ExitStack() as d_stack:
            wod_pool = d_stack.enter_context(tc.tile_pool(name="wod", bufs=2))
            o_pool = d_stack.enter_context(tc.tile_pool(name="op", bufs=4))

            for dt in range(NDT):
                wod = wod_pool.tile([128, QH, DT], bf, name="wod")
                # scalar HWDGE queue: keeps Wo loads off the sync queue, which
                # is busy streaming the output tiles back to DRAM
                nc.scalar.dma_start(out=wod, in_=wo_r[:, :, dt * DT : (dt + 1) * DT])
                for sc in range(NSC):
                    po = psA.tile([128, DT], f32, name="acc")
                    for h in range(QH):
                        nc.tensor.matmul(
                            po,
                            ctxr[:, h, sc * SC : (sc + 1) * SC],
                            wod[:, h, :],
                            start=(h == 0),
                            stop=(h == QH - 1),
                        )
                    o_sb = o_pool.tile([128, DT], bf, name="osb")
                    if sc % 2 == 0:
                        nc.scalar.copy(o_sb, po)
                    else:
                        nc.vector.tensor_copy(o_sb, po)
                    nc.sync.dma_start(
                        out=out[sc * SC : (sc + 1) * SC, dt * DT : (dt + 1) * DT],
                        in_=o_sb,
                    )

    _legalize_waits(nc)
    return nc


_NC_CACHE = {}
_last_exec_ns = None


def _get_nc():
    if "nc" not in _NC_CACHE:
        _NC_CACHE["nc"] = _build_nc()
    return _NC_CACHE["nc"]


# ---------------------------------------------------------------------------
# Optional NTFF profiling hook (used by the local test harness via
# KERNEL_TRACE=1; grading path leaves it off)
# ---------------------------------------------------------------------------
def _install_ntff_hook(so_path="/opt/axon/libaxon_pjrt.so"):
    if "antenv.axon_hooks" in sys.modules:
        return
    try:
        lib = ctypes.CDLL(so_path)
    except OSError:
        lib = None
    if lib is None or not hasattr(lib, "axon_start_nrt_profile"):
        hook = None
    else:
        lib.axon_start_nrt_profile.argtypes = [
            ctypes.POINTER(ctypes.c_int64),
            ctypes.c_size_t,
        ]
        lib.axon_start_nrt_profile.restype = ctypes.c_int64
        lib.axon_stop_nrt_profile.argtypes = [ctypes.c_char_p]
        lib.axon_stop_nrt_profile.restype = ctypes.c_int64

        @contextlib.contextmanager
        def hook(output_dir, device_ids):
            import jax

            jax.devices()
            if device_ids:
                ids = (ctypes.c_int64 * len(device_ids))(*device_ids)
                rc = lib.axon_start_nrt_profile(ids, len(device_ids))
            else:
                rc = lib.axon_start_nrt_profile(None, 0)
            if rc != 0:
                raise RuntimeError(f"axon_start_nrt_profile rc={rc}")
            try:
                yield
            finally:
                n = lib.axon_stop_nrt_profile(str(output_dir).encode())
                print(f"ntff profile: {n} file(s) -> {output_dir}", file=sys.stderr)

    mod = types.ModuleType("antenv.axon_hooks")
    mod.get_axon_ntff_profile_hook = lambda: hook
    sys.modules["antenv.axon_hooks"] = mod


# ---------------------------------------------------------------------------
# Host entry point
# ---------------------------------------------------------------------------
def kernel(hidden_states, position_ids, attention_mask, Wq, Wk, Wv, Wo):
    global _last_exec_ns
    from concourse import bass_utils

    hidden_states = np.asarray(hidden_states, dtype=np.float32)
    position_ids = np.asarray(position_ids)
    attention_mask = np.asarray(attention_mask)
    Wq = np.asarray(Wq, dtype=np.float32)
    Wk = np.asarray(Wk, dtype=np.float32)
    Wv = np.asarray(Wv, dtype=np.float32)
    Wo = np.asarray(Wo, dtype=np.float32)

    if not np.all(np.asarray(attention_mask) > 0):
        # Spec guarantees an all-ones mask; fall back to a host reference
        # implementation for the general case rather than mis-computing.
        return _host_reference(
            hidden_states, position_ids, attention_mask, Wq, Wk, Wv, Wo
        )

    # rope tables per batch: cc/ss [HD, S] with halves stacked
    half = HD // 2
    inv_freq = 1.0 / (THETA ** (np.arange(0, half, dtype=np.float32) / half))
    ccs, sss = [], []
    for b in range(B):
        freqs = position_ids[b].astype(np.float32)[:, None] * inv_freq[None, :]
        cosT = np.cos(freqs).T.astype(np.float32)  # [64, S]
        sinT = np.sin(freqs).T.astype(np.float32)
        ccs.append(
            np.ascontiguousarray(np.concatenate([cosT, cosT], axis=0).astype(BF16))
        )
        sss.append(
            np.ascontiguousarray(np.concatenate([sinT, sinT], axis=0).astype(BF16))
        )

    # diagonal-block 0/1 masks: block t: dmask[k, t*QT + q] = (q >= t*SC + k)
    kk = np.arange(SC)[:, None]
    qq = np.arange(QT)[None, :]
    dmask = np.concatenate(
        [(qq >= t * SC + kk).astype(np.float32) for t in range(G)], axis=1
    ).astype(BF16)
    dmask = np.ascontiguousarray(dmask)

    # head-selector stationaries for softmax row sums / broadcast
    sel8 = np.zeros((SC, QH * QH), dtype=np.float32)
    for h in range(QH):
        sel8[:, h * QH + h] = 1.0
    sel8 = np.ascontiguousarray(sel8.astype(BF16))
    selbc = np.zeros((QH, QH * SC), dtype=np.float32)
    for h in range(QH):
        selbc[h, h * SC : (h + 1) * SC] = 1.0
    selbc = np.ascontiguousarray(selbc.astype(BF16))

    hiTs = [np.ascontiguousarray(hidden_states[b].T.astype(BF16)) for b in range(B)]

    in_maps = []
    for c in range(NCORES):
        b = c // KV_SHARDS
        m = c % KV_SHARDS
        qcols = slice(m * FQ, (m + 1) * FQ)
        kvcols = slice(m * FKV, (m + 1) * FKV)
        in_maps.append(
            {
                "hiT": hiTs[b],
                "wq": np.ascontiguousarray(Wq[:, qcols].astype(BF16)),
                "wk": np.ascontiguousarray(Wk[:, kvcols].astype(BF16)),
                "wv": np.ascontiguousarray(Wv[:, kvcols].astype(BF16)),
                "wo": np.ascontiguousarray(Wo[qcols, :].astype(BF16)),
                "ccT": ccs[b],
                "ssT": sss[b],
                "dmask": dmask,
                "sel8": sel8,
                "selbc": selbc,
            }
        )

    nc = _get_nc()
    trace = os.environ.get("KERNEL_TRACE", "") == "1"
    if trace:
        _install_ntff_hook()
        bass_utils.upload_artifacts = lambda tmpdir: f"local:{tmpdir}"
    res = bass_utils.run_bass_kernel_spmd(
        nc, in_maps, list(range(NCORES)), trace=trace
    )
    _last_exec_ns = res.exec_time_ns

    out = np.zeros((B, S, D), dtype=np.float32)
    for c in range(NCORES):
        out[c // KV_SHARDS] += np.asarray(res.results[c]["out"]).astype(np.float32)
    return out


def _host_reference(hidden_states, position_ids, attention_mask, Wq, Wk, Wv, Wo):
    """Numpy fallback for inputs outside the spec's guarantees."""
    q = (hidden_states @ Wq).reshape(B, S, H, HD)
    k = (hidden_states @ Wk).reshape(B, S, HKV, HD)
    v = (hidden_states @ Wv).reshape(B, S, HKV, HD)

    half = HD // 2
    inv_freq = 1.0 / (THETA ** (np.arange(0, half, dtype=np.float32) / half))
    freqs = position_ids.astype(np.float32)[..., None] * inv_freq
    cos = np.cos(freqs)[:, :, None, :]
    sin = np.sin(freqs)[:, :, None, :]

    def rope(x):
        x1, x2 = x[..., :half], x[..., half:]
        return np.concatenate([x1 * cos - x2 * sin, x2 * cos + x1 * sin], axis=-1)

    q, k = rope(q), rope(k)
    qg = q.reshape(B, S, HKV, G, HD)
    scores = np.einsum("bqhgd,bkhd->bhgqk", qg, k) * (HD**-0.5)
    causal = np.tril(np.ones((S, S), bool))
    mask = causal[None, None, None] & (attention_mask[:, None, None, None, :] > 0)
    scores = np.where(mask, scores, np.finfo(np.float32).min)
    scores = scores - scores.max(axis=-1, keepdims=True)
    probs = np.exp(scores)
    probs = probs / probs.sum(axis=-1, keepdims=True)
    ctx = np.einsum("bhgqk,bkhd->bqhgd", probs, v).reshape(B, S, H * HD)
    return (ctx @ Wo).astype(np.float32)


# revision 27
# speedup vs baseline: 1.5472x; 1.0355x over previous
"""Trainium2 Bass kernel for nn_ExaoneAttention (dense transformer attention).

Full-input contract: kernel(**inputs) takes the unsharded inputs and returns
the full [B, S, D] output. Internally shards across 8 NeuronCores:
2-way data parallel over batch x 4-way tensor parallel over kv heads
(2 kv heads = 8 query heads per core). Each core computes a partial
output through its Wo row-slice; the host sums the 4 partials per batch.

v2 design (bf16 operands, f32 PSUM accumulation):
- Single fused pass: for each 512-token tile, project K/V/Q (weights
  stationary, hidden-state tile moving), rope on the fly, then run
  attention for that query tile against all keys so far. Output
  projection runs as a tail phase with Wo streamed per 512-column block.
  No DRAM staging round-trips; K/V/ctx live in SBUF for the whole kernel.
- Softmax in the "scoresT" orientation (keys on partitions, queries on
  the free dim). Per-chunk exp runs on 1024-wide PSUM pairs. Row sums
  accumulate via matmuls with a head-selector stationary into one
  [8, 512] PSUM tile per query tile, giving a single batched reciprocal;
  the reciprocal row is broadcast back over partitions with a tiny
  matmul and multiplied into the unnormalized context in place.
- Causal diagonal blocks are handled multiplicatively: exp first, then
  a 0/1 bf16 mask multiply (2x DVE throughput).
"""

import contextlib
import ctypes
import os
import sys
import types

import ml_dtypes
import numpy as np

# ---------------------------------------------------------------------------
# Problem constants (hardcoded per contract)
# ---------------------------------------------------------------------------
B, S, D = 2, 2048, 4096
H, HKV, HD = 32, 8, 128
G = H // HKV
THETA = 10000.0

NCORES = 8
BAT_SHARDS = 2
KV_SHARDS = 4
KVH = HKV // KV_SHARDS  # kv heads per core = 2
QH = KVH * G  # q heads per core = 8
FQ = QH * HD  # 1024
FKV = KVH * HD  # 256
DMC = D // 128  # 32 model-dim chunks

QT = 512  # query tile
NQT = S // QT  # 4
SC = 128  # sequence chunk
NSC = S // SC  # 16
DT = 512  # output d tile
NDT = D // DT  # 8

_SCALE = float(HD) ** -0.5
BF16 = ml_dtypes.bfloat16


# ---------------------------------------------------------------------------
# Wait-count legalization: this walrus build rejects instructions carrying
# more than a small number of sync waits (fused fp32/fp32r matmul: >1;
# drain: >4). Hoist excess waits onto standalone NoOps on the same engine
# immediately before the offending instruction; AND-semantics are preserved
# by sequential same-engine execution.
# ---------------------------------------------------------------------------
def _legalize_waits(nc):
    import bass_rust
    import concourse.mybir as mybir

    counter = 0
    for f in nc.m.functions:
        for bb in f.blocks:
            il = bb.instructions
            i = 0
            while i < len(il):
                ins = il[i]
                si = ins.sync_info
                if si is None or len(si.on_wait) <= 1:
                    i += 1
                    continue
                waits = list(si.on_wait)
                pos = i
                for w in waits[1:]:
                    counter += 1
                    nop = mybir.InstNoOp(name=f"lgw-{counter}", ins=[], outs=[])
                    nop.engine = ins.engine
                    nop.sync_info = bass_rust.SyncInfo(on_wait=[w], on_update=[])
                    il.insert(pos, nop)
                    pos += 1
                    i += 1
                ins.sync_info = bass_rust.SyncInfo(
                    on_wait=waits[:1], on_update=list(si.on_update)
                )
                i += 1
    return counter


# ---------------------------------------------------------------------------
# Bass kernel builder (per-core program; same program on all 8 cores)
# ---------------------------------------------------------------------------
def _build_nc():
    import concourse.bass as bass
    import concourse.mybir as mybir
    from concourse.masks import make_identity
    from concourse.tile import TileContext

    f32 = mybir.dt.float32
    bf = mybir.dt.bfloat16
    AF = mybir.ActivationFunctionType

    nc = bass.Bass()

    # All tensors are host-pre-packed so every DMA is contiguous per
    # partition (strided gathers ran at ~1/4 bandwidth and dominated the
    # kernel's warm-up).
    NSLAB = 4
    hiT = nc.declare_dram_parameter(
        "hiT", [NQT, NSLAB, 128, (DMC // NSLAB) * QT], bf, isOutput=False
    )
    wq = nc.declare_dram_parameter("wq", [G, 128, DMC * 2 * HD], bf, isOutput=False)
    wk = nc.declare_dram_parameter("wk", [128, DMC * FKV], bf, isOutput=False)
    wv = nc.declare_dram_parameter("wv", [128, DMC * FKV], bf, isOutput=False)
    wo = nc.declare_dram_parameter("wo", [NDT, 128, QH * DT], bf, isOutput=False)
    ccT = nc.declare_dram_parameter("ccT", [HD, S], bf, isOutput=False)
    ssT = nc.declare_dram_parameter("ssT", [HD, S], bf, isOutput=False)
    dmask = nc.declare_dram_parameter("dmask", [SC, 4 * QT], bf, isOutput=False)
    sel8 = nc.declare_dram_parameter("sel8", [SC, QH * QH], bf, isOutput=False)
    selbc = nc.declare_dram_parameter("selbc", [QH, QH * SC], bf, isOutput=False)
    out = nc.declare_dram_parameter("out", [NDT, NSC, SC, DT], bf, isOutput=True)

    with TileContext(nc) as tc, contextlib.ExitStack() as top:
        const_pool = top.enter_context(tc.tile_pool(name="const", bufs=1))
        persist = top.enter_context(tc.tile_pool(name="persist", bufs=1))
        psA = top.enter_context(tc.tile_pool(name="psA", bufs=4, space="PSUM"))
        psB = top.enter_context(tc.tile_pool(name="psB", bufs=2, space="PSUM"))
        norm_pool = top.enter_context(tc.tile_pool(name="norm", bufs=2))

        # wk leads the sync queue (the first K-pass needs it); everything
        # else rides the Activation HWDGE queue in parallel.
        wk_sb = const_pool.tile([128, DMC, FKV], bf)
        nc.sync.dma_start(out=wk_sb, in_=wk[:, :])
        wv_sb = const_pool.tile([128, DMC, FKV], bf)
        nc.scalar.dma_start(out=wv_sb, in_=wv[:, :])
        cc_sb = const_pool.tile([HD, S], bf)
        nc.scalar.dma_start(out=cc_sb, in_=ccT[:, :])
        ss_sb = const_pool.tile([HD, S], bf)
        nc.scalar.dma_start(out=ss_sb, in_=ssT[:, :])
        dm_sb = const_pool.tile([SC, 4 * QT], bf)
        nc.scalar.dma_start(out=dm_sb, in_=dmask[:, :])
        sel8_sb = const_pool.tile([SC, QH * QH], bf)
        nc.scalar.dma_start(out=sel8_sb, in_=sel8[:, :])
        selbc_sb = const_pool.tile([QH, QH * SC], bf)
        nc.scalar.dma_start(out=selbc_sb, in_=selbc[:, :])

        ident = const_pool.tile([128, 128], bf)
        make_identity(nc, ident)

        # persistent activation stores
        ktr = persist.tile([128, KVH, S], bf)  # roped K^T  [d, kv, s]
        vt = persist.tile([128, KVH, NSC, HD], bf)  # V  [s-chunk, kv, sc, d]
        ctxr = persist.tile([128, QH, S], bf)  # ctx^T  [d, h, s]

        with contextlib.ExitStack() as proj_stack:
            slab_pool = proj_stack.enter_context(tc.tile_pool(name="slab", bufs=4))
            wqp_pool = proj_stack.enter_context(tc.tile_pool(name="wqp", bufs=2))
            stage_pool = proj_stack.enter_context(tc.tile_pool(name="stage", bufs=4))
            qcur_pool = proj_stack.enter_context(tc.tile_pool(name="qcur", bufs=2))
            pt_pool = proj_stack.enter_context(tc.tile_pool(name="pt", bufs=4))

            def rope_evac(pk, dest, ssl):
                """dest[:, :] = rope(pk) in bf16; dest is a [128, QT] AP.

                The partition-shifted multiplies read the PSUM accumulator
                directly: the walrus verifier only allows mismatched base
                partitions when the operands are in different memory spaces.
                """
                x = stage_pool.tile([128, QT], bf, name="ropex")
                nc.scalar.copy(x, pk)
                t1 = stage_pool.tile([128, QT], bf, name="ropet1")
                nc.vector.tensor_mul(t1, x, cc_sb[:, ssl])
                t2 = stage_pool.tile([128, QT], bf, name="ropet2")
                nc.vector.tensor_mul(t2[:64], pk[64:], ss_sb[:64, ssl])
                nc.vector.tensor_mul(t2[64:], pk[:64], ss_sb[64:, ssl])
                nc.vector.tensor_sub(dest[:64], t1[:64], t2[:64])
                nc.vector.tensor_add(dest[64:], t1[64:], t2[64:])

            def normalize(pred_sb, ssl_p):
                """probs denominators -> 1/sum, broadcast, scale ctx in place."""
                recip = norm_pool.tile([QH, QT], bf, name="recip")
                with nc.allow_low_precision(reason="bf16 softmax recip"):
                    nc.vector.reciprocal(recip, pred_sb)
                for h in range(QH):
                    pbc = psA.tile([128, QT], f32, name="acc")
                    nc.tensor.matmul(
                        pbc,
                        selbc_sb[:, h * SC : (h + 1) * SC],
                        recip,
                        start=True,
                        stop=True,
                    )
                    nc.vector.tensor_mul(
                        ctxr[:, h, ssl_p], ctxr[:, h, ssl_p], pbc
                    )

            pending = None  # deferred (pred_sb, ssl) from the previous tile
            for st in range(NQT):
                ssl = slice(st * QT, (st + 1) * QT)
                slabs = []
                for hh in range(NSLAB):
                    slab = slab_pool.tile([128, DMC // NSLAB, QT], bf, name="slab")
                    nc.sync.dma_start(out=slab, in_=hiT[st, hh])
                    slabs.append(slab)

                qcur = qcur_pool.tile([128, QH, QT], bf, name="qcur")

                # 6 projection passes of 2 PSUM banks each:
                # (k0,k1), (v0,v1), (q0,q1), (q2,q3), (q4,q5), (q6,q7)
                passes = [("kv", 0), ("kv", 1)] + [("q", j) for j in range(4)]
                for kind, pj in passes:
                    if kind == "q":
                        wqp = wqp_pool.tile([128, DMC, 2 * HD], bf, name="wqp")
                        nc.sync.dma_start(out=wqp, in_=wq[pj])
                    banks = [
                        psA.tile([128, QT], f32, name="acc") for _ in range(2)
                    ]
                    for c in range(DMC):
                        mv = slabs[c // (DMC // NSLAB)][:, c % (DMC // NSLAB), :]
                        for j in range(2):
                            if kind == "kv":
                                wsrc = wk_sb if pj == 0 else wv_sb
                                stat = wsrc[:, c, j * 128 : (j + 1) * 128]
                            else:
                                stat = wqp[:, c, j * 128 : (j + 1) * 128]
                            nc.tensor.matmul(
                                banks[j],
                                stat,
                                mv,
                                start=(c == 0),
                                stop=(c == DMC - 1),
                            )
                    if kind == "kv" and pj == 0:  # K heads
                        for j in range(2):
                            rope_evac(banks[j], ktr[:, j, ssl], ssl)
                    elif kind == "kv":  # V heads
                        for j in range(2):
                            v_sb = stage_pool.tile([128, QT], bf, name="vstage")
                            nc.scalar.copy(v_sb, banks[j])
                            for t in range(QT // 128):
                                ptr = psA.tile([128, 128], bf, name="acc")
                                nc.tensor.transpose(
                                    ptr, v_sb[:, t * 128 : (t + 1) * 128], ident
                                )
                                nc.vector.tensor_copy(
                                    vt[:, j, st * 4 + t, :], ptr
                                )
                    else:  # Q pair
                        for j in range(2):
                            rope_evac(banks[j], qcur[:, pj * 2 + j, :], ssl)

                # previous tile's softmax normalization: emitted here so its
                # PE work (broadcast matmuls) sits behind this tile's
                # projection matmuls, hiding the DVE reciprocal latency
                if pending is not None:
                    normalize(*pending)
                    pending = None

                # ---------------- attention for query tile st ----------------
                nk = G * (st + 1)
                npair = nk // 2
                pred = psA.tile([QH, QT], f32, name="acc")
                for h in range(QH):
                    kv = h // G
                    pctx = psA.tile([128, QT], f32, name="acc")

                    def flush_pv(pt_p):
                        """PV + row-sum matmuls for a softmaxed pair."""
                        pt_f, p_f = pt_p
                        for half in range(2):
                            i = 2 * p_f + half
                            ph = pt_f[:, half * QT : (half + 1) * QT]
                            nc.tensor.matmul(
                                pctx,
                                vt[:, kv, i, :],
                                ph,
                                start=(i == 0),
                                stop=(i == nk - 1),
                            )
                            nc.tensor.matmul(
                                pred,
                                sel8_sb[:, h * QH : (h + 1) * QH],
                                ph,
                                start=(h == 0 and i == 0),
                                stop=(h == QH - 1 and i == nk - 1),
                            )

                    # software-pipelined with lag 2: the PV matmuls for pair p
                    # are emitted after the scores of pair p+2, so the PE never
                    # waits on the ACT exp of the pair it is about to consume
                    ptq = []
                    for p in range(npair):
                        pp = psB.tile([128, 2 * QT], f32, name="pair")
                        for half in range(2):
                            i = 2 * p + half
                            nc.tensor.matmul(
                                pp[:, half * QT : (half + 1) * QT],
                                ktr[:, kv, i * SC : (i + 1) * SC],
                                qcur[:, h, :],
                                start=True,
                                stop=True,
                            )
                        pt = pt_pool.tile([128, 2 * QT], bf, name="pt")
                        nc.scalar.activation(pt, pp, AF.Exp, scale=_SCALE)
                        if p >= npair - 2:  # diagonal pair -> 0/1 mask
                            t = p - (npair - 2)
                            nc.vector.tensor_mul(
                                pt, pt, dm_sb[:, t * 2 * QT : (t + 1) * 2 * QT]
                            )
                        ptq.append((pt, p))
                        if len(ptq) > 2:
                            flush_pv(ptq.pop(0))
                    for pt_p in ptq:
                        flush_pv(pt_p)
                    # unnormalized ctx -> SBUF (normalized in place later)
                    nc.scalar.copy(ctxr[:, h, ssl], pctx)

                # free the PSUM bank; the reciprocal + broadcast run after the
                # next tile's projection passes
                pred_sb = norm_pool.tile([QH, QT], f32, name="predsb")
                nc.scalar.copy(pred_sb, pred)
                pending = (pred_sb, ssl)

        # ---------------- output projection (Wo streamed per d-tile) --------
        # The last tile's normalize is folded in after the first 12 s-chunks
        # of dt=0 (which only touch already-normalized context), so the PE
        # never waits on the final reciprocal.
        with contextlib.ExitStack() as d_stack:
            wod_pool = d_stack.enter_context(tc.tile_pool(name="wod", bufs=2))
            o_pool = d_stack.enter_context(tc.tile_pool(name="op", bufs=4))

            for dt in range(NDT):
                wod = wod_pool.tile([128, QH, DT], bf, name="wod")
                # scalar HWDGE queue: keeps Wo loads off the sync queue, which
                # is busy streaming the output tiles back to DRAM
                nc.scalar.dma_start(out=wod, in_=wo[dt])
                for sc in range(NSC):
                    if pending is not None and (dt > 0 or sc >= 3 * NSC // 4):
                        normalize(*pending)
                        pending = None
                    po = psA.tile([128, DT], f32, name="acc")
                    for h in range(QH):
                        nc.tensor.matmul(
                            po,
                            ctxr[:, h, sc * SC : (sc + 1) * SC],
                            wod[:, h, :],
                            start=(h == 0),
                            stop=(h == QH - 1),
                        )
                    o_sb = o_pool.tile([128, DT], bf, name="osb")
                    if sc % 2 == 0:
                        nc.scalar.copy(o_sb, po)
                    else:
                        nc.vector.tensor_copy(o_sb, po)
                    nc.sync.dma_start(out=out[dt, sc], in_=o_sb)

    _legalize_waits(nc)
    return nc


_NC_CACHE = {}
_last_exec_ns = None


def _get_nc():
    if "nc" not in _NC_CACHE:
        _NC_CACHE["nc"] = _build_nc()
    return _NC_CACHE["nc"]


# ---------------------------------------------------------------------------
# Optional NTFF profiling hook (used by the local test harness via
# KERNEL_TRACE=1; grading path leaves it off)
# ---------------------------------------------------------------------------
def _install_ntff_hook(so_path="/opt/axon/libaxon_pjrt.so"):
    if "antenv.axon_hooks" in sys.modules:
        return
    try:
        lib = ctypes.CDLL(so_path)
    except OSError:
        lib = None
    if lib is None or not hasattr(lib, "axon_start_nrt_profile"):
        hook = None
    else:
        lib.axon_start_nrt_profile.argtypes = [
            ctypes.POINTER(ctypes.c_int64),
            ctypes.c_size_t,
        ]
        lib.axon_start_nrt_profile.restype = ctypes.c_int64
        lib.axon_stop_nrt_profile.argtypes = [ctypes.c_char_p]
        lib.axon_stop_nrt_profile.restype = ctypes.c_int64

        @contextlib.contextmanager
        def hook(output_dir, device_ids):
            import jax

            jax.devices()
            if device_ids:
                ids = (ctypes.c_int64 * len(device_ids))(*device_ids)
                rc = lib.axon_start_nrt_profile(ids, len(device_ids))
            else:
                rc = lib.axon_start_nrt_profile(None, 0)
            if rc != 0:
                raise RuntimeError(f"axon_start_nrt_profile rc={rc}")
            try:
                yield
            finally:
                n = lib.axon_stop_nrt_profile(str(output_dir).encode())
                print(f"ntff profile: {n} file(s) -> {output_dir}", file=sys.stderr)

    mod = types.ModuleType("antenv.axon_hooks")
    mod.get_axon_ntff_profile_hook = lambda: hook
    sys.modules["antenv.axon_hooks"] = mod


# ---------------------------------------------------------------------------
# Host entry point
# ---------------------------------------------------------------------------
def kernel(hidden_states, position_ids, attention_mask, Wq, Wk, Wv, Wo):
    global _last_exec_ns
    from concourse import bass_utils

    hidden_states = np.asarray(hidden_states, dtype=np.float32)
    position_ids = np.asarray(position_ids)
    attention_mask = np.asarray(attention_mask)
    Wq = np.asarray(Wq, dtype=np.float32)
    Wk = np.asarray(Wk, dtype=np.float32)
    Wv = np.asarray(Wv, dtype=np.float32)
    Wo = np.asarray(Wo, dtype=np.float32)

    if not np.all(np.asarray(attention_mask) > 0):
        # Spec guarantees an all-ones mask; fall back to a host reference
        # implementation for the general case rather than mis-computing.
        return _host_reference(
            hidden_states, position_ids, attention_mask, Wq, Wk, Wv, Wo
        )

    # rope tables per batch: cc/ss [HD, S] with halves stacked
    half = HD // 2
    inv_freq = 1.0 / (THETA ** (np.arange(0, half, dtype=np.float32) / half))
    ccs, sss = [], []
    for b in range(B):
        freqs = position_ids[b].astype(np.float32)[:, None] * inv_freq[None, :]
        cosT = np.cos(freqs).T.astype(np.float32)  # [64, S]
        sinT = np.sin(freqs).T.astype(np.float32)
        ccs.append(
            np.ascontiguousarray(np.concatenate([cosT, cosT], axis=0).astype(BF16))
        )
        sss.append(
            np.ascontiguousarray(np.concatenate([sinT, sinT], axis=0).astype(BF16))
        )

    # diagonal-block 0/1 masks: block t: dmask[k, t*QT + q] = (q >= t*SC + k)
    kk = np.arange(SC)[:, None]
    qq = np.arange(QT)[None, :]
    dmask = np.concatenate(
        [(qq >= t * SC + kk).astype(np.float32) for t in range(G)], axis=1
    ).astype(BF16)
    dmask = np.ascontiguousarray(dmask)

    # head-selector stationaries for softmax row sums / broadcast
    sel8 = np.zeros((SC, QH * QH), dtype=np.float32)
    for h in range(QH):
        sel8[:, h * QH + h] = 1.0
    sel8 = np.ascontiguousarray(sel8.astype(BF16))
    selbc = np.zeros((QH, QH * SC), dtype=np.float32)
    for h in range(QH):
        selbc[h, h * SC : (h + 1) * SC] = 1.0
    selbc = np.ascontiguousarray(selbc.astype(BF16))

    # pre-packed, per-partition-contiguous DRAM layouts (see _build_nc)
    NSLAB = 4

    def pack_hi(b):
        # [st, hh, p, cc*512+u] = hidden[b].T[(hh*8+cc)*128+p, st*512+u]
        c1 = hidden_states[b].T.astype(BF16).reshape(DMC, 128, NQT, QT)
        return np.ascontiguousarray(
            c1.reshape(NSLAB, DMC // NSLAB, 128, NQT, QT)
            .transpose(3, 0, 2, 1, 4)
            .reshape(NQT, NSLAB, 128, (DMC // NSLAB) * QT)
        )

    def pack_kv(w):  # [p, c*256+f] = w[c*128+p, f]
        return np.ascontiguousarray(
            w.astype(BF16).reshape(DMC, 128, FKV).transpose(1, 0, 2).reshape(128, -1)
        )

    def pack_wq(w):  # [pj, p, c*256 + j*128 + v] = w[c*128+p, pj*256+j*128+v]
        return np.ascontiguousarray(
            w.astype(BF16)
            .reshape(DMC, 128, G, 2 * HD)
            .transpose(2, 1, 0, 3)
            .reshape(G, 128, DMC * 2 * HD)
        )

    def pack_wo(w):  # [dt, p, h*512+u] = w[h*128+p, dt*512+u]
        return np.ascontiguousarray(
            w.astype(BF16)
            .reshape(QH, 128, NDT, DT)
            .transpose(2, 1, 0, 3)
            .reshape(NDT, 128, QH * DT)
        )

    hiTs = [pack_hi(b) for b in range(B)]

    in_maps = []
    for c in range(NCORES):
        b = c // KV_SHARDS
        m = c % KV_SHARDS
        qcols = slice(m * FQ, (m + 1) * FQ)
        kvcols = slice(m * FKV, (m + 1) * FKV)
        in_maps.append(
            {
                "hiT": hiTs[b],
                "wq": pack_wq(Wq[:, qcols]),
                "wk": pack_kv(Wk[:, kvcols]),
                "wv": pack_kv(Wv[:, kvcols]),
                "wo": pack_wo(Wo[qcols, :]),
                "ccT": ccs[b],
                "ssT": sss[b],
                "dmask": dmask,
                "sel8": sel8,
                "selbc": selbc,
            }
        )

    nc = _get_nc()
    trace = os.environ.get("KERNEL_TRACE", "") == "1"
    if trace:
        _install_ntff_hook()
        bass_utils.upload_artifacts = lambda tmpdir: f"local:{tmpdir}"
    res = bass_utils.run_bass_kernel_spmd(
        nc, in_maps, list(range(NCORES)), trace=trace
    )
    _last_exec_ns = res.exec_time_ns

    out = np.zeros((B, S, D), dtype=np.float32)
    for c in range(NCORES):
        # unblock [dt, sc, p, u] -> [sc*128+p, dt*512+u]
        blk = np.asarray(res.results[c]["out"]).astype(np.float32)
        out[c // KV_SHARDS] += blk.transpose(1, 2, 0, 3).reshape(S, D)
    return out


def _host_reference(hidden_states, position_ids, attention_mask, Wq, Wk, Wv, Wo):
    """Numpy fallback for inputs outside the spec's guarantees."""
    q = (hidden_states @ Wq).reshape(B, S, H, HD)
    k = (hidden_states @ Wk).reshape(B, S, HKV, HD)
    v = (hidden_states @ Wv).reshape(B, S, HKV, HD)

    half = HD // 2
    inv_freq = 1.0 / (THETA ** (np.arange(0, half, dtype=np.float32) / half))
    freqs = position_ids.astype(np.float32)[..., None] * inv_freq
    cos = np.cos(freqs)[:, :, None, :]
    sin = np.sin(freqs)[:, :, None, :]

    def rope(x):
        x1, x2 = x[..., :half], x[..., half:]
        return np.concatenate([x1 * cos - x2 * sin, x2 * cos + x1 * sin], axis=-1)

    q, k = rope(q), rope(k)
    qg = q.reshape(B, S, HKV, G, HD)
    scores = np.einsum("bqhgd,bkhd->bhgqk", qg, k) * (HD**-0.5)
    causal = np.tril(np.ones((S, S), bool))
    mask = causal[None, None, None] & (attention_mask[:, None, None, None, :] > 0)
    scores = np.where(mask, scores, np.finfo(np.float32).min)
    scores = scores - scores.max(axis=-1, keepdims=True)
    probs = np.exp(scores)
    probs = probs / probs.sum(axis=-1, keepdims=True)
    ctx = np.einsum("bhgqk,bkhd->bqhgd", probs, v).reshape(B, S, H * HD)
    return (ctx @ Wo).astype(np.float32)


# revision 33
# speedup vs baseline: 1.5508x; 1.0023x over previous
"""Trainium2 Bass kernel for nn_ExaoneAttention (dense transformer attention).

Full-input contract: kernel(**inputs) takes the unsharded inputs and returns
the full [B, S, D] output. Internally shards across 8 NeuronCores:
2-way data parallel over batch x 4-way tensor parallel over kv heads
(2 kv heads = 8 query heads per core). Each core computes a partial
output through its Wo row-slice; the host sums the 4 partials per batch.

v2 design (bf16 operands, f32 PSUM accumulation):
- Single fused pass: for each 512-token tile, project K/V/Q (weights
  stationary, hidden-state tile moving), rope on the fly, then run
  attention for that query tile against all keys so far. Output
  projection runs as a tail phase with Wo streamed per 512-column block.
  No DRAM staging round-trips; K/V/ctx live in SBUF for the whole kernel.
- Softmax in the "scoresT" orientation (keys on partitions, queries on
  the free dim). Per-chunk exp runs on 1024-wide PSUM pairs. Row sums
  accumulate via matmuls with a head-selector stationary into one
  [8, 512] PSUM tile per query tile, giving a single batched reciprocal;
  the reciprocal row is broadcast back over partitions with a tiny
  matmul and multiplied into the unnormalized context in place.
- Causal diagonal blocks are handled multiplicatively: exp first, then
  a 0/1 bf16 mask multiply (2x DVE throughput).
"""

import contextlib
import ctypes
import os
import sys
import types

import ml_dtypes
import numpy as np

# ---------------------------------------------------------------------------
# Problem constants (hardcoded per contract)
# ---------------------------------------------------------------------------
B, S, D = 2, 2048, 4096
H, HKV, HD = 32, 8, 128
G = H // HKV
THETA = 10000.0

NCORES = 8
BAT_SHARDS = 2
KV_SHARDS = 4
KVH = HKV // KV_SHARDS  # kv heads per core = 2
QH = KVH * G  # q heads per core = 8
FQ = QH * HD  # 1024
FKV = KVH * HD  # 256
DMC = D // 128  # 32 model-dim chunks

QT = 512  # query tile
NQT = S // QT  # 4
SC = 128  # sequence chunk
NSC = S // SC  # 16
DT = 512  # output d tile
NDT = D // DT  # 8

_SCALE = float(HD) ** -0.5
BF16 = ml_dtypes.bfloat16


# ---------------------------------------------------------------------------
# Wait-count legalization: this walrus build rejects instructions carrying
# more than a small number of sync waits (fused fp32/fp32r matmul: >1;
# drain: >4). Hoist excess waits onto standalone NoOps on the same engine
# immediately before the offending instruction; AND-semantics are preserved
# by sequential same-engine execution.
# ---------------------------------------------------------------------------
def _legalize_waits(nc):
    import bass_rust
    import concourse.mybir as mybir

    counter = 0
    for f in nc.m.functions:
        for bb in f.blocks:
            il = bb.instructions
            i = 0
            while i < len(il):
                ins = il[i]
                si = ins.sync_info
                if si is None or len(si.on_wait) <= 1:
                    i += 1
                    continue
                waits = list(si.on_wait)
                pos = i
                for w in waits[1:]:
                    counter += 1
                    nop = mybir.InstNoOp(name=f"lgw-{counter}", ins=[], outs=[])
                    nop.engine = ins.engine
                    nop.sync_info = bass_rust.SyncInfo(on_wait=[w], on_update=[])
                    il.insert(pos, nop)
                    pos += 1
                    i += 1
                ins.sync_info = bass_rust.SyncInfo(
                    on_wait=waits[:1], on_update=list(si.on_update)
                )
                i += 1
    return counter


# ---------------------------------------------------------------------------
# Bass kernel builder (per-core program; same program on all 8 cores)
# ---------------------------------------------------------------------------
def _build_nc():
    import concourse.bass as bass
    import concourse.mybir as mybir
    from concourse.masks import make_identity
    from concourse.tile import TileContext

    f32 = mybir.dt.float32
    bf = mybir.dt.bfloat16
    AF = mybir.ActivationFunctionType

    nc = bass.Bass()

    # All tensors are host-pre-packed so every DMA is contiguous per
    # partition (strided gathers ran at ~1/4 bandwidth and dominated the
    # kernel's warm-up).
    NSLAB = 4
    hiT = nc.declare_dram_parameter(
        "hiT", [NQT, NSLAB, 128, (DMC // NSLAB) * QT], bf, isOutput=False
    )
    wq = nc.declare_dram_parameter("wq", [G, 128, DMC * 2 * HD], bf, isOutput=False)
    wk = nc.declare_dram_parameter("wk", [128, DMC * FKV], bf, isOutput=False)
    wv = nc.declare_dram_parameter("wv", [128, DMC * FKV], bf, isOutput=False)
    wo = nc.declare_dram_parameter("wo", [NDT, 128, QH * DT], bf, isOutput=False)
    ccT = nc.declare_dram_parameter("ccT", [HD, S], bf, isOutput=False)
    ssT = nc.declare_dram_parameter("ssT", [HD, S], bf, isOutput=False)
    dmask = nc.declare_dram_parameter("dmask", [SC, 4 * QT], bf, isOutput=False)
    sel8 = nc.declare_dram_parameter("sel8", [SC, QH * QH], bf, isOutput=False)
    selbc = nc.declare_dram_parameter("selbc", [QH, QH * SC], bf, isOutput=False)
    perm64 = nc.declare_dram_parameter("perm64", [SC, SC], bf, isOutput=False)
    out = nc.declare_dram_parameter("out", [NDT, NSC, SC, DT], bf, isOutput=True)

    with TileContext(nc) as tc, contextlib.ExitStack() as top:
        const_pool = top.enter_context(tc.tile_pool(name="const", bufs=1))
        persist = top.enter_context(tc.tile_pool(name="persist", bufs=1))
        psA = top.enter_context(tc.tile_pool(name="psA", bufs=4, space="PSUM"))
        psB = top.enter_context(tc.tile_pool(name="psB", bufs=2, space="PSUM"))
        norm_pool = top.enter_context(tc.tile_pool(name="norm", bufs=2))

        # The first K-pass needs exactly wk + the first hidden slab (4MB).
        # They lead the two HWDGE queues so they get the full HBM bandwidth;
        # every other transfer is queued behind them.
        wk_sb = const_pool.tile([128, DMC, FKV], bf)
        nc.sync.dma_start(out=wk_sb, in_=wk[:, :])
        wv_sb = const_pool.tile([128, DMC, FKV], bf)
        cc_sb = const_pool.tile([HD, S], bf)
        ss_sb = const_pool.tile([HD, S], bf)
        dm_sb = const_pool.tile([SC, 4 * QT], bf)
        sel8_sb = const_pool.tile([SC, QH * QH], bf)
        selbc_sb = const_pool.tile([QH, QH * SC], bf)
        perm_sb = const_pool.tile([SC, SC], bf)

        def _load_tables():
            nc.scalar.dma_start(out=wv_sb, in_=wv[:, :])
            nc.scalar.dma_start(out=cc_sb, in_=ccT[:, :])
            nc.scalar.dma_start(out=ss_sb, in_=ssT[:, :])
            nc.scalar.dma_start(out=dm_sb, in_=dmask[:, :])
            nc.scalar.dma_start(out=sel8_sb, in_=sel8[:, :])
            nc.scalar.dma_start(out=selbc_sb, in_=selbc[:, :])
            nc.scalar.dma_start(out=perm_sb, in_=perm64[:, :])

        ident = const_pool.tile([128, 128], bf)
        make_identity(nc, ident)

        # persistent activation stores
        ktr = persist.tile([128, KVH, S], bf)  # roped K^T  [d, kv, s]
        vt = persist.tile([128, KVH, NSC, HD], bf)  # V  [s-chunk, kv, sc, d]
        ctxr = persist.tile([128, QH, S], bf)  # ctx^T  [d, h, s]

        with contextlib.ExitStack() as proj_stack:
            slab_pool = proj_stack.enter_context(tc.tile_pool(name="slab", bufs=4))
            wqp_pool = proj_stack.enter_context(tc.tile_pool(name="wqp", bufs=2))
            stage_pool = proj_stack.enter_context(tc.tile_pool(name="stage", bufs=4))
            qcur_pool = proj_stack.enter_context(tc.tile_pool(name="qcur", bufs=2))
            pt_pool = proj_stack.enter_context(tc.tile_pool(name="pt", bufs=4))

            def rope_evac(pk, dest, ssl):
                """dest[:, :] = rope(pk) in bf16; dest is a [128, QT] AP.

                rotate_half runs as a PE permutation matmul so the projection
                PSUM bank is released by the single ACT copy; the sin table
                carries the rotation signs ([-sin; +sin]), leaving a short
                mul/mul/add chain on the DVE.
                """
                x = stage_pool.tile([128, QT], bf, name="ropex")
                nc.scalar.copy(x, pk)
                xp = psA.tile([128, QT], f32, name="acc")
                nc.tensor.matmul(xp, perm_sb, x, start=True, stop=True)
                t1 = stage_pool.tile([128, QT], bf, name="ropet1")
                nc.vector.tensor_mul(t1, x, cc_sb[:, ssl])
                t2 = stage_pool.tile([128, QT], bf, name="ropet2")
                nc.vector.tensor_mul(t2, xp, ss_sb[:, ssl])
                nc.vector.tensor_add(dest, t1, t2)

            def normalize(pred_sb, ssl_p):
                """probs denominators -> 1/sum, broadcast, scale ctx in place."""
                recip = norm_pool.tile([QH, QT], bf, name="recip")
                with nc.allow_low_precision(reason="bf16 softmax recip"):
                    nc.vector.reciprocal(recip, pred_sb)
                for h in range(QH):
                    pbc = psA.tile([128, QT], f32, name="acc")
                    nc.tensor.matmul(
                        pbc,
                        selbc_sb[:, h * SC : (h + 1) * SC],
                        recip,
                        start=True,
                        stop=True,
                    )
                    nc.vector.tensor_mul(
                        ctxr[:, h, ssl_p], ctxr[:, h, ssl_p], pbc
                    )

            pending = None  # deferred (pred_sb, ssl) from the previous tile
            for st in range(NQT):
                ssl = slice(st * QT, (st + 1) * QT)
                slabs = []
                for hh in range(NSLAB):
                    slab = slab_pool.tile([128, DMC // NSLAB, QT], bf, name="slab")
                    # first slab rides the scalar queue, concurrent with wk
                    eng = nc.scalar if (st == 0 and hh == 0) else nc.sync
                    eng.dma_start(out=slab, in_=hiT[st, hh])
                    slabs.append(slab)
                if st == 0:
                    _load_tables()

                qcur = qcur_pool.tile([128, QH, QT], bf, name="qcur")

                # 6 projection passes of 2 PSUM banks each:
                # (k0,k1), (v0,v1), (q0,q1), (q2,q3), (q4,q5), (q6,q7)
                passes = [("kv", 0), ("kv", 1)] + [("q", j) for j in range(4)]
                for kind, pj in passes:
                    if kind == "q":
                        wqp = wqp_pool.tile([128, DMC, 2 * HD], bf, name="wqp")
                        nc.sync.dma_start(out=wqp, in_=wq[pj])
                    banks = [
                        psA.tile([128, QT], f32, name="acc") for _ in range(2)
                    ]
                    for c in range(DMC):
                        mv = slabs[c // (DMC // NSLAB)][:, c % (DMC // NSLAB), :]
                        for j in range(2):
                            if kind == "kv":
                                wsrc = wk_sb if pj == 0 else wv_sb
                                stat = wsrc[:, c, j * 128 : (j + 1) * 128]
                            else:
                                stat = wqp[:, c, j * 128 : (j + 1) * 128]
                            nc.tensor.matmul(
                                banks[j],
                                stat,
                                mv,
                                start=(c == 0),
                                stop=(c == DMC - 1),
                            )
                    if kind == "kv" and pj == 0:  # K heads
                        for j in range(2):
                            rope_evac(banks[j], ktr[:, j, ssl], ssl)
                    elif kind == "kv":  # V heads
                        for j in range(2):
                            v_sb = stage_pool.tile([128, QT], bf, name="vstage")
                            nc.scalar.copy(v_sb, banks[j])
                            for t in range(QT // 128):
                                ptr = psA.tile([128, 128], bf, name="acc")
                                nc.tensor.transpose(
                                    ptr, v_sb[:, t * 128 : (t + 1) * 128], ident
                                )
                                nc.vector.tensor_copy(
                                    vt[:, j, st * 4 + t, :], ptr
                                )
                    else:  # Q pair
                        for j in range(2):
                            rope_evac(banks[j], qcur[:, pj * 2 + j, :], ssl)

                # previous tile's softmax normalization: emitted here so its
                # PE work (broadcast matmuls) sits behind this tile's
                # projection matmuls, hiding the DVE reciprocal latency
                if pending is not None:
                    normalize(*pending)
                    pending = None

                # ---------------- attention for query tile st ----------------
                nk = G * (st + 1)
                npair = nk // 2
                pred = psA.tile([QH, QT], f32, name="acc")
                for h in range(QH):
                    kv = h // G
                    pctx = psA.tile([128, QT], f32, name="acc")

                    def flush_pv(pt_p):
                        """PV + row-sum matmuls for a softmaxed pair."""
                        pt_f, p_f = pt_p
                        for half in range(2):
                            i = 2 * p_f + half
                            ph = pt_f[:, half * QT : (half + 1) * QT]
                            nc.tensor.matmul(
                                pctx,
                                vt[:, kv, i, :],
                                ph,
                                start=(i == 0),
                                stop=(i == nk - 1),
                            )
                            nc.tensor.matmul(
                                pred,
                                sel8_sb[:, h * QH : (h + 1) * QH],
                                ph,
                                start=(h == 0 and i == 0),
                                stop=(h == QH - 1 and i == nk - 1),
                            )

                    # software-pipelined with lag 2: the PV matmuls for pair p
                    # are emitted after the scores of pair p+2, so the PE never
                    # waits on the ACT exp of the pair it is about to consume
                    ptq = []
                    for p in range(npair):
                        pp = psB.tile([128, 2 * QT], f32, name="pair")
                        for half in range(2):
                            i = 2 * p + half
                            nc.tensor.matmul(
                                pp[:, half * QT : (half + 1) * QT],
                                ktr[:, kv, i * SC : (i + 1) * SC],
                                qcur[:, h, :],
                                start=True,
                                stop=True,
                            )
                        pt = pt_pool.tile([128, 2 * QT], bf, name="pt")
                        nc.scalar.activation(pt, pp, AF.Exp, scale=_SCALE)
                        if p >= npair - 2:  # diagonal pair -> 0/1 mask
                            t = p - (npair - 2)
                            nc.vector.tensor_mul(
                                pt, pt, dm_sb[:, t * 2 * QT : (t + 1) * 2 * QT]
                            )
                        ptq.append((pt, p))
                        if len(ptq) > 2:
                            flush_pv(ptq.pop(0))
                    for pt_p in ptq:
                        flush_pv(pt_p)
                    # unnormalized ctx -> SBUF (normalized in place later)
                    nc.scalar.copy(ctxr[:, h, ssl], pctx)

                # free the PSUM bank; the reciprocal + broadcast run after the
                # next tile's projection passes
                pred_sb = norm_pool.tile([QH, QT], f32, name="predsb")
                nc.scalar.copy(pred_sb, pred)
                pending = (pred_sb, ssl)

        # ---------------- output projection (Wo streamed per d-tile) --------
        # The last tile's normalize is folded in after the first 12 s-chunks
        # of dt=0 (which only touch already-normalized context), so the PE
        # never waits on the final reciprocal.
        with contextlib.ExitStack() as d_stack:
            wod_pool = d_stack.enter_context(tc.tile_pool(name="wod", bufs=2))
            o_pool = d_stack.enter_context(tc.tile_pool(name="op", bufs=4))

            for dt in range(NDT):
                wod = wod_pool.tile([128, QH, DT], bf, name="wod")
                # scalar HWDGE queue: keeps Wo loads off the sync queue, which
                # is busy streaming the output tiles back to DRAM
                nc.scalar.dma_start(out=wod, in_=wo[dt])
                for sc in range(NSC):
                    if pending is not None and (dt > 0 or sc >= 3 * NSC // 4):
                        normalize(*pending)
                        pending = None
                    po = psA.tile([128, DT], f32, name="acc")
                    for h in range(QH):
                        nc.tensor.matmul(
                            po,
                            ctxr[:, h, sc * SC : (sc + 1) * SC],
                            wod[:, h, :],
                            start=(h == 0),
                            stop=(h == QH - 1),
                        )
                    o_sb = o_pool.tile([128, DT], bf, name="osb")
                    if sc % 2 == 0:
                        nc.scalar.copy(o_sb, po)
                    else:
                        nc.vector.tensor_copy(o_sb, po)
                    nc.sync.dma_start(out=out[dt, sc], in_=o_sb)

    _legalize_waits(nc)
    return nc


_NC_CACHE = {}
_last_exec_ns = None


def _get_nc():
    if "nc" not in _NC_CACHE:
        _NC_CACHE["nc"] = _build_nc()
    return _NC_CACHE["nc"]


# ---------------------------------------------------------------------------
# Optional NTFF profiling hook (used by the local test harness via
# KERNEL_TRACE=1; grading path leaves it off)
# ---------------------------------------------------------------------------
def _install_ntff_hook(so_path="/opt/axon/libaxon_pjrt.so"):
    if "antenv.axon_hooks" in sys.modules:
        return
    try:
        lib = ctypes.CDLL(so_path)
    except OSError:
        lib = None
    if lib is None or not hasattr(lib, "axon_start_nrt_profile"):
        hook = None
    else:
        lib.axon_start_nrt_profile.argtypes = [
            ctypes.POINTER(ctypes.c_int64),
            ctypes.c_size_t,
        ]
        lib.axon_start_nrt_profile.restype = ctypes.c_int64
        lib.axon_stop_nrt_profile.argtypes = [ctypes.c_char_p]
        lib.axon_stop_nrt_profile.restype = ctypes.c_int64

        @contextlib.contextmanager
        def hook(output_dir, device_ids):
            import jax

            jax.devices()
            if device_ids:
                ids = (ctypes.c_int64 * len(device_ids))(*device_ids)
                rc = lib.axon_start_nrt_profile(ids, len(device_ids))
            else:
                rc = lib.axon_start_nrt_profile(None, 0)
            if rc != 0:
                raise RuntimeError(f"axon_start_nrt_profile rc={rc}")
            try:
                yield
            finally:
                n = lib.axon_stop_nrt_profile(str(output_dir).encode())
                print(f"ntff profile: {n} file(s) -> {output_dir}", file=sys.stderr)

    mod = types.ModuleType("antenv.axon_hooks")
    mod.get_axon_ntff_profile_hook = lambda: hook
    sys.modules["antenv.axon_hooks"] = mod


# ---------------------------------------------------------------------------
# Host entry point
# ---------------------------------------------------------------------------
def kernel(hidden_states, position_ids, attention_mask, Wq, Wk, Wv, Wo):
    global _last_exec_ns
    from concourse import bass_utils

    hidden_states = np.asarray(hidden_states, dtype=np.float32)
    position_ids = np.asarray(position_ids)
    attention_mask = np.asarray(attention_mask)
    Wq = np.asarray(Wq, dtype=np.float32)
    Wk = np.asarray(Wk, dtype=np.float32)
    Wv = np.asarray(Wv, dtype=np.float32)
    Wo = np.asarray(Wo, dtype=np.float32)

    if not np.all(np.asarray(attention_mask) > 0):
        # Spec guarantees an all-ones mask; fall back to a host reference
        # implementation for the general case rather than mis-computing.
        return _host_reference(
            hidden_states, position_ids, attention_mask, Wq, Wk, Wv, Wo
        )

    # rope tables per batch: cc/ss [HD, S] with halves stacked
    half = HD // 2
    inv_freq = 1.0 / (THETA ** (np.arange(0, half, dtype=np.float32) / half))
    ccs, sss = [], []
    for b in range(B):
        freqs = position_ids[b].astype(np.float32)[:, None] * inv_freq[None, :]
        cosT = np.cos(freqs).T.astype(np.float32)  # [64, S]
        sinT = np.sin(freqs).T.astype(np.float32)
        ccs.append(
            np.ascontiguousarray(np.concatenate([cosT, cosT], axis=0).astype(BF16))
        )
        sss.append(
            np.ascontiguousarray(np.concatenate([sinT, sinT], axis=0).astype(BF16))
        )

    # diagonal-block 0/1 masks: block t: dmask[k, t*QT + q] = (q >= t*SC + k)
    kk = np.arange(SC)[:, None]
    qq = np.arange(QT)[None, :]
    dmask = np.concatenate(
        [(qq >= t * SC + kk).astype(np.float32) for t in range(G)], axis=1
    ).astype(BF16)
    dmask = np.ascontiguousarray(dmask)

    # head-selector stationaries for softmax row sums / broadcast
    sel8 = np.zeros((SC, QH * QH), dtype=np.float32)
    for h in range(QH):
        sel8[:, h * QH + h] = 1.0
    sel8 = np.ascontiguousarray(sel8.astype(BF16))
    selbc = np.zeros((QH, QH * SC), dtype=np.float32)
    for h in range(QH):
        selbc[h, h * SC : (h + 1) * SC] = 1.0
    selbc = np.ascontiguousarray(selbc.astype(BF16))

    # pre-packed, per-partition-contiguous DRAM layouts (see _build_nc)
    NSLAB = 4

    def pack_hi(b):
        # [st, hh, p, cc*512+u] = hidden[b].T[(hh*8+cc)*128+p, st*512+u]
        c1 = hidden_states[b].T.astype(BF16).reshape(DMC, 128, NQT, QT)
        return np.ascontiguousarray(
            c1.reshape(NSLAB, DMC // NSLAB, 128, NQT, QT)
            .transpose(3, 0, 2, 1, 4)
            .reshape(NQT, NSLAB, 128, (DMC // NSLAB) * QT)
        )

    def pack_kv(w):  # [p, c*256+f] = w[c*128+p, f]
        return np.ascontiguousarray(
            w.astype(BF16).reshape(DMC, 128, FKV).transpose(1, 0, 2).reshape(128, -1)
        )

    def pack_wq(w):  # [pj, p, c*256 + j*128 + v] = w[c*128+p, pj*256+j*128+v]
        return np.ascontiguousarray(
            w.astype(BF16)
            .reshape(DMC, 128, G, 2 * HD)
            .transpose(2, 1, 0, 3)
            .reshape(G, 128, DMC * 2 * HD)
        )

    def pack_wo(w):  # [dt, p, h*512+u] = w[h*128+p, dt*512+u]
        return np.ascontiguousarray(
            w.astype(BF16)
            .reshape(QH, 128, NDT, DT)
            .transpose(2, 1, 0, 3)
            .reshape(NDT, 128, QH * DT)
        )

    hiTs = [pack_hi(b) for b in range(B)]

    in_maps = []
    for c in range(NCORES):
        b = c // KV_SHARDS
        m = c % KV_SHARDS
        qcols = slice(m * FQ, (m + 1) * FQ)
        kvcols = slice(m * FKV, (m + 1) * FKV)
        in_maps.append(
            {
                "hiT": hiTs[b],
                "wq": pack_wq(Wq[:, qcols]),
                "wk": pack_kv(Wk[:, kvcols]),
                "wv": pack_kv(Wv[:, kvcols]),
                "wo": pack_wo(Wo[qcols, :]),
                "ccT": ccs[b],
                "ssT": sss[b],
                "dmask": dmask,
                "sel8": sel8,
                "selbc": selbc,
            }
        )

    nc = _get_nc()
    trace = os.environ.get("KERNEL_TRACE", "") == "1"
    if trace:
        _install_ntff_hook()
        bass_utils.upload_artifacts = lambda tmpdir: f"local:{tmpdir}"
    res = bass_utils.run_bass_kernel_spmd(
        nc, in_maps, list(range(NCORES)), trace=trace
    )
    _last_exec_ns = res.exec_time_ns

    out = np.zeros((B, S, D), dtype=np.float32)
    for c in range(NCORES):
        # unblock [dt, sc, p, u] -> [sc*128+p, dt*512+u]
        blk = np.asarray(res.results[c]["out"]).astype(np.float32)
        out[c // KV_SHARDS] += blk.transpose(1, 2, 0, 3).reshape(S, D)
    return out


def _host_reference(hidden_states, position_ids, attention_mask, Wq, Wk, Wv, Wo):
    """Numpy fallback for inputs outside the spec's guarantees."""
    q = (hidden_states @ Wq).reshape(B, S, H, HD)
    k = (hidden_states @ Wk).reshape(B, S, HKV, HD)
    v = (hidden_states @ Wv).reshape(B, S, HKV, HD)

    half = HD // 2
    inv_freq = 1.0 / (THETA ** (np.arange(0, half, dtype=np.float32) / half))
    freqs = position_ids.astype(np.float32)[..., None] * inv_freq
    cos = np.cos(freqs)[:, :, None, :]
    sin = np.sin(freqs)[:, :, None, :]

    def rope(x):
        x1, x2 = x[..., :half], x[..., half:]
        return np.concatenate([x1 * cos - x2 * sin, x2 * cos + x1 * sin], axis=-1)

    q, k = rope(q), rope(k)
    qg = q.reshape(B, S, HKV, G, HD)
    scores = np.einsum("bqhgd,bkhd->bhgqk", qg, k) * (HD**-0.5)
    causal = np.tril(np.ones((S, S), bool))
    mask = causal[None, None, None] & (attention_mask[:, None, None, None, :] > 0)
    scores = np.where(mask, scores, np.finfo(np.float32).min)
    scores = scores - scores.max(axis=-1, keepdims=True)
    probs = np.exp(scores)
    probs = probs / probs.sum(axis=-1, keepdims=True)
    ctx = np.einsum("bhgqk,bkhd->bqhgd", probs, v).reshape(B, S, H * HD)
    return (ctx @ Wo).astype(np.float32)
